# revision 1
# baseline (speedup 1.0000x reference)
"""Multi-head causal attention (B=2, S=2048, D=1024, H=16) on 8 TRN2 NeuronCores.

Sharding: batch*head parallel. Core c handles batch b = c//4 and the 4
heads h in [4*(c%4), 4*(c%4)+4). Each core computes its heads' Q/K/V
projections (column-parallel), causal softmax attention, and its partial
row-parallel output projection; the host sums the 4 partial outputs per
batch (the AllReduce of row-parallel tensor parallelism).

On-device layout: everything is kept "transposed" (feature-major) so
every matmul contracts along the partition dimension:
  scoresT[k,q] = K Q^T      (per head, 128-row k-tiles x 512-col q-tiles)
  P^T = exp(scoresT/8 + mask/8)   (additive -1e9 causal mask, PE-accumulated)
  outT[d,q]   = sum_k V[k,d] P^T[k,q]   (PSUM-accumulated over k-tiles)
  sums[q]     = sum_k P^T[k,q]          (ones-vector matmul, col-packed)
  y[q,e]     += sum_hd outT_norm[hd,q] * w_oT[hd,e]
Softmax skips the max-subtraction: scores ~ N(0,1), so exp never
overflows fp32, and exp(-1e9/8) underflows to exactly 0 like the
reference's masked_fill(-1e9).

Matmuls run as float32r (TF32-like, 1 cycle/row at N>=512; measured
~1.5e-4 rms per matmul). Fully-masked 128x512 blocks are skipped
(causal => ~62% of blocks computed).
"""

import numpy as np

D_MODEL = 1024
N_HEADS = 16
D_K = 64
B, S = 2, 2048
N_CORES = 8
HPC = 4            # heads per core
KT = S // 128      # 16 k-tiles
QT = S // 512      # 4 q-tiles
ET = D_MODEL // 128  # 8 e-tiles (contraction tiles for projections)

ATT_BF16 = False  # bf16 scores/attnV matmuls (f32r projections + output proj)
ET_BF16 = True   # bf16 exp output + V operand for the attnV matmul only

_PROG_CACHE = {}


def _build_program():
    import concourse.bacc as bacc_mod
    import concourse.mybir as mybir
    import concourse.tile as tile

    f32 = mybir.dt.float32
    f32r = mybir.dt.float32r
    bf16 = mybir.dt.bfloat16
    att_dt = bf16 if ATT_BF16 else f32r
    et_dt = bf16 if (ATT_BF16 or ET_BF16) else f32r
    Exp = mybir.ActivationFunctionType.Exp

    nc = bacc_mod.Bacc(
        "TRN2", target_bir_lowering=False, debug=False, num_devices=N_CORES
    )

    xq = nc.dram_tensor("xq", [D_MODEL, S], f32r, kind="ExternalInput").ap()
    xk = nc.dram_tensor("xk", [D_MODEL, S], f32r, kind="ExternalInput").ap()
    xv = nc.dram_tensor("xv", [D_MODEL, S], f32r, kind="ExternalInput").ap()
    wq = nc.dram_tensor("wq", [D_MODEL, 256], f32r, kind="ExternalInput").ap()
    wk = nc.dram_tensor("wk", [D_MODEL, 256], f32r, kind="ExternalInput").ap()
    wv = nc.dram_tensor("wv", [D_MODEL, 256], f32r, kind="ExternalInput").ap()
    wo = nc.dram_tensor("wo", [256, D_MODEL], f32r, kind="ExternalInput").ap()
    maskt = nc.dram_tensor("maskt", [128, 2048], mybir.dt.bfloat16, kind="ExternalInput").ap()
    idbf = nc.dram_tensor("idbf", [128, 132], mybir.dt.bfloat16, kind="ExternalInput").ap()
    consts = nc.dram_tensor("consts", [128, 193], f32r, kind="ExternalInput").ap()
    y = nc.dram_tensor("y", [S, D_MODEL], f32, kind="ExternalOutput").ap()

    with (
        tile.TileContext(nc) as tc,
        nc.allow_low_precision("fp32r attention"),
        tc.tile_pool(name="persist", bufs=1) as pp,
    ):
        # ---- persistent SBUF tiles ----
        def persist(shape, dtype, name):
            return pp.tile(shape, dtype, name=name, tag=name)

        wq_sb = persist([128, ET * 256], f32r, "wq_sb")
        wk_sb = persist([128, ET * 256], f32r, "wk_sb")
        wv_sb = persist([128, ET * 256], f32r, "wv_sb")
        wo_sb = [persist([128, D_MODEL], f32r, f"wo_sb{p}") for p in range(2)]
        maskt_sb = persist([128, 2048], mybir.dt.bfloat16, "maskt_sb")
        idbf_sb = persist([128, 132], mybir.dt.bfloat16, "idbf_sb")
        consts_sb = persist([128, 193], f32r, "consts_sb")
        qt_sb = [persist([128, S], att_dt, f"qt_sb{p}") for p in range(2)]
        kt_sb = [persist([128, S], att_dt, f"kt_sb{p}") for p in range(2)]
        v_sb = [persist([128, 260], et_dt, f"v_sb{i}") for i in range(KT)]
        outt_sb = [persist([128, S], f32r, f"outt_sb{p}") for p in range(2)]

        identity = consts_sb[:, 0:128]
        ones_col = consts_sb[:, 128:192]   # [128, 64] of 1.0
        ones1 = consts_sb[:, 192:193]      # [128, 1] of 1.0

        # small consts first (the PE warm-up pack depends on them)
        nc.sync.dma_start(out=consts_sb[:], in_=consts[:])
        nc.sync.dma_start(out=idbf_sb[:], in_=idbf[:])
        nc.sync.dma_start(out=maskt_sb[:], in_=maskt[:])
        # weight loads: [1024, 256] -> [128, 8*256] (e-tile t at cols 256t)
        for w_dram, w_tile in ((wq, wq_sb), (wk, wk_sb), (wv, wv_sb)):
            nc.sync.dma_start(
                out=w_tile[:].rearrange("p (t d) -> p t d", t=ET),
                in_=w_dram.rearrange("(t p) d -> p t d", p=128),
            )
        for p in range(2):
            nc.sync.dma_start(out=wo_sb[p][:], in_=wo[p * 128 : (p + 1) * 128, :])

        # ---- PE warm-up ----
        # The PE HAM clock gate starts (and re-enters) K=4/8 half-clock and
        # only returns to full clock after ~3.4us of gapless PE activity.
        # Dense same-stationary dummy matmuls (results never read) force the
        # transition; packs are re-issued wherever the schedule has an
        # unavoidable multi-us PE idle (DMA-bound ramp, phase boundaries,
        # softmax-normalize tails).
        def emit_warm_pack(pool, count, tag="warm", name="warm"):
            wt = pool.tile([128, 512], f32, name=name, tag=tag)
            for w in range(count):
                nc.tensor.matmul(
                    wt[:],
                    idbf_sb[:, 0:128],
                    maskt_sb[:, 0:512],
                    start=True,
                    stop=True,
                )

        with tc.tile_pool(name="psW", bufs=1, space="PSUM") as psW:
            emit_warm_pack(psW, 24, name="warm_start")

        # ---- phase B: projections ----
        # Q^T/K^T accumulate over all 8 e-tiles into [128, 2048] PSUM (8
        # banks, both m-tiles). The strided xv DMAs are emitted interleaved
        # with the xq/xk streams so the V-projection (which must wait for
        # the QK PSUM banks anyway) starts with its data already resident
        # and runs as a dense PE burst instead of being DMA-paced.
        with (
            tc.tile_pool(name="xe", bufs=3) as xep,
            tc.tile_pool(name="xvk", bufs=10) as xvkp,
        ):
            vdma_tiles = []

            def emit_v_dma():
                i = len(vdma_tiles)
                xvk = xvkp.tile([128, ET * 128], f32r, name=f"xvk_{i}", tag="xvk")
                nc.sync.dma_start(
                    out=xvk[:].rearrange("p (t k) -> p t k", t=ET),
                    in_=xv[:, i * 128 : (i + 1) * 128].rearrange(
                        "(t p) k -> p t k", p=128
                    ),
                )
                vdma_tiles.append(xvk)

            psA_ctx = tc.tile_pool(name="psA", bufs=1, space="PSUM")
            psA = psA_ctx.__enter__()
            for ti, (x_dram, w_tile, dst) in enumerate(
                ((xq, wq_sb, qt_sb), (xk, wk_sb, kt_sb))
            ):
                ps = [
                    psA.tile(
                        [128, S], f32, name=f"ps_p{ti}_{m}", tag=f"proj{m}", bufs=1
                    )
                    for m in range(2)
                ]
                for e in range(ET):
                    xe = xep.tile([128, S], f32r, name=f"xe_{ti}_{e}", tag="xe")
                    nc.sync.dma_start(out=xe[:], in_=x_dram[e * 128 : (e + 1) * 128, :])
                    if ti == 1 or e >= 6:
                        emit_v_dma()
                    for m in range(2):
                        lhsT = w_tile[:, e * 256 + m * 128 : e * 256 + (m + 1) * 128]
                        for n in range(QT):
                            nc.tensor.matmul(
                                ps[m][:, n * 512 : (n + 1) * 512],
                                lhsT,
                                xe[:, n * 512 : (n + 1) * 512],
                                start=(e == 0),
                                stop=(e == ET - 1),
                            )
                for m in range(2):
                    nc.vector.tensor_copy(dst[m][:], ps[m][:])

            psA_ctx.__exit__(None, None, None)
            psV_ctx = tc.tile_pool(name="psV", bufs=2, space="PSUM")
            psV = psV_ctx.__enter__()
            # V projection: dense burst (data already largely resident)
            for i in range(KT):
                if i >= len(vdma_tiles) - 2 and len(vdma_tiles) < KT:
                    emit_v_dma()
                psv = psV.tile([128, 256], f32, name=f"psv_{i}", tag="v")
                xvk = vdma_tiles[i]
                for e in range(ET):
                    nc.tensor.matmul(
                        psv[:],
                        xvk[:, e * 128 : (e + 1) * 128],
                        wv_sb[:, e * 256 : (e + 1) * 256],
                        start=(e == 0),
                        stop=(e == ET - 1),
                    )
                nc.vector.tensor_copy(
                    v_sb[i][:].rearrange("p (h c) -> p h c", c=65)[:, :, 0:64],
                    psv[:].rearrange("p (h d) -> p h d", d=64),
                )
                ones4 = idbf_sb[:, 128:132] if (ATT_BF16 or ET_BF16) else consts_sb[:, 128:132]
                nc.vector.tensor_copy(
                    v_sb[i][:].rearrange("p (h c) -> p h c", c=65)[:, :, 64:65],
                    ones4.rearrange("p (h c) -> p h c", c=1),
                )
            while len(vdma_tiles) < KT:
                emit_v_dma()
            psV_ctx.__exit__(None, None, None)

        # ---- phase C+D: attention with interleaved output projection ----
        # One head-pair per pass (pr = 0, 1). Per (pr, j): score tiles are
        # [128, 1024] head-pair PSUM tiles (row-packed score MMs fill the two
        # banks concurrently; ONE exp per round at FD=1024 runs ~2x faster
        # per element). attnV accumulates into a [65, 1024] pair tile (row
        # 64 = sum of exp via the ones column of v_sb). Normalization of
        # q-block j-1 is emitted lazily inside block j so its DVE chain and
        # broadcast matmuls never stall the PE; the output projection of
        # block j-1 runs as dense filler inside the pr=1 pass.
        with (
            tc.tile_pool(name="psS", bufs=3, space="PSUM") as psS,
            tc.tile_pool(name="psO", bufs=1, space="PSUM") as psO,
            tc.tile_pool(name="et", bufs=6) as etp,
            tc.tile_pool(name="bcsb", bufs=3) as bcp,
            tc.tile_pool(name="rcsb", bufs=3) as rcp,
            tc.tile_pool(name="ysb", bufs=3) as ysbp,
        ):
            def emit_outproj_mtile(m):
                psy = psS.tile([128, 1024], f32, name=f"psy_{m}", tag="s")
                for p in range(2):
                    for n in range(2):
                        nc.tensor.matmul(
                            psy[:, n * 512 : (n + 1) * 512],
                            outt_sb[p][:, m * 128 : (m + 1) * 128],
                            wo_sb[p][:, n * 512 : (n + 1) * 512],
                            start=(p == 0),
                            stop=(p == 1),
                        )
                y_sb = ysbp.tile([128, 1024], f32, name=f"y_sb_{m}", tag="ysb")
                nc.vector.tensor_copy(y_sb[:], psy[:])
                nc.sync.dma_start(out=y[m * 128 : (m + 1) * 128, :], in_=y_sb[:])

            def emit_normalize(pr, jj, ps_out_prev):
                qsj = slice(jj * 512, (jj + 1) * 512)
                ssb = rcp.tile([33, 512], f32, name=f"ssb_{pr}_{jj}", tag="ssb")
                for hh in range(2):
                    nc.vector.tensor_copy(
                        ssb[32 * hh : 32 * hh + 1, :],
                        ps_out_prev[64:65, 512 * hh : 512 * (hh + 1)],
                    )
                rc32 = rcp.tile([33, 512], f32, name=f"rc32_{pr}_{jj}", tag="rc32")
                nc.vector.reciprocal_approx_fast(out=rc32[:], in_=ssb[:])
                rc = rcp.tile([33, 512], f32r, name=f"rc_{pr}_{jj}", tag="rc")
                nc.vector.tensor_copy(rc[:], rc32[:])
                bc = psS.tile([128, 1024], f32, name=f"ps_bc_{pr}_{jj}", tag="s")
                for hh in range(2):
                    nc.tensor.matmul(
                        bc[0:64, 512 * hh : 512 * (hh + 1)],
                        consts_sb[32 * hh : 32 * hh + 1, 128:192],
                        rc[32 * hh : 32 * hh + 1, :],
                        start=True,
                        stop=True,
                        tile_position=(32 * hh, 0),
                    )
                bc_sb = bcp.tile([64, 1024], f32, name=f"bc_sb_{pr}_{jj}", tag="bc")
                nc.vector.tensor_copy(bc_sb[:], bc[0:64, :])
                for hh in range(2):
                    nc.vector.tensor_mul(
                        outt_sb[pr][64 * hh : 64 * hh + 64, qsj],
                        ps_out_prev[0:64, 512 * hh : 512 * (hh + 1)],
                        bc_sb[:, 512 * hh : 512 * (hh + 1)],
                    )

            for pr in range(2):
                pending_norm = None  # (pr, j, ps_out) awaiting lazy normalize
                pending_out = None   # q-block awaiting output projection (pr=1)
                j_order = range(QT) if pr == 0 else range(QT - 1, -1, -1)
                for j in j_order:
                    n_i = 4 * j + 4
                    qs = slice(j * 512, (j + 1) * 512)
                    ps_out = psO.tile(
                        [65, 1024], f32, name=f"ps_out_{pr}_{j}", tag="o"
                    )
                    prev_et = None
                    prev_i = -1
                    for i in range(n_i):
                        diag = i >= 4 * j
                        r = i - 4 * j
                        pss = psS.tile(
                            [128, 1024], f32, name=f"ps_s{pr}_{j}_{i}", tag="s"
                        )
                        if diag:
                            nw = 128 * (r + 1)
                            for hh in range(2):
                                nc.tensor.matmul(
                                    pss[:, 512 * hh : 512 * hh + nw],
                                    idbf_sb[:, 0:128],
                                    maskt_sb[:, r * 512 : r * 512 + nw],
                                    start=True,
                                    stop=False,
                                )
                        for hh in range(2):
                            hp = slice(64 * hh, 64 * hh + 64)
                            nc.tensor.matmul(
                                pss[:, 512 * hh : 512 * (hh + 1)],
                                kt_sb[pr][hp, i * 128 : (i + 1) * 128],
                                qt_sb[pr][hp, qs],
                                start=not diag,
                                stop=True,
                            )
                        et = etp.tile(
                            [128, 1024], et_dt, name=f"et{pr}_{j}_{i}", tag="et"
                        )
                        nc.scalar.activation(et[:], pss[:], Exp, scale=0.125)
                        if prev_et is not None:
                            for hh in range(2):
                                nc.tensor.matmul(
                                    ps_out[:, 512 * hh : 512 * (hh + 1)],
                                    v_sb[prev_i][:, (2 * pr + hh) * 65 : (2 * pr + hh + 1) * 65],
                                    prev_et[:, 512 * hh : 512 * (hh + 1)],
                                    start=(prev_i == 0),
                                    stop=(prev_i == n_i - 1),
                                )
                        prev_et, prev_i = et, i
                        if i == 1 and pending_norm is not None:
                            pn_j = pending_norm[1]
                            emit_normalize(*pending_norm)
                            pending_norm = None
                            if pr == 1:
                                pending_out = pn_j
                        if pending_out is not None and i == 2:
                            for m in range(4 * pending_out, 4 * pending_out + 4):
                                emit_outproj_mtile(m)
                            pending_out = None
                    for hh in range(2):
                        nc.tensor.matmul(
                            ps_out[:, 512 * hh : 512 * (hh + 1)],
                            v_sb[n_i - 1][:, (2 * pr + hh) * 65 : (2 * pr + hh + 1) * 65],
                            prev_et[:, 512 * hh : 512 * (hh + 1)],
                            start=(n_i - 1 == 0),
                            stop=True,
                        )
                    pending_norm = (pr, j, ps_out)
                emit_normalize(*pending_norm)
                if pr == 1:
                    if pending_out is not None:
                        for m in range(4 * pending_out, 4 * pending_out + 4):
                            emit_outproj_mtile(m)
                    # pr1 runs j descending, so the pass ends on j=0
                    for m in range(0, 4):
                        emit_outproj_mtile(m)

    nc.compile()
    return nc


def _get_program():
    if "nc" not in _PROG_CACHE:
        _PROG_CACHE["nc"] = _build_program()
    return _PROG_CACHE["nc"]


def _host_prep(query, key, value, mask, w_q, w_k, w_v, w_o):
    query = np.asarray(query, dtype=np.float32)
    key = np.asarray(key, dtype=np.float32)
    value = np.asarray(value, dtype=np.float32)
    w_q = np.asarray(w_q, dtype=np.float32)
    w_k = np.asarray(w_k, dtype=np.float32)
    w_v = np.asarray(w_v, dtype=np.float32)
    w_o = np.asarray(w_o, dtype=np.float32)
    m = np.asarray(mask).reshape(S, S).astype(bool)

    # The kernel's block-skip structure assumes the standard causal mask.
    expected = np.triu(np.ones((S, S), dtype=bool), k=1)
    if not np.array_equal(m, expected):
        raise NotImplementedError("kernel specialized for causal (triu, k=1) mask")

    # 4 canonical diagonal-straddle mask tiles: pattern r covers k-tile
    # 4j+r vs q-tile j; masked where (128r + row) > col.
    import ml_dtypes

    maskt = np.zeros((128, 2048), dtype=np.float32)
    rows = np.arange(128)[:, None]
    cols = np.arange(512)[None, :]
    for r in range(4):
        maskt[:, r * 512 : (r + 1) * 512] = np.where(
            (128 * r + rows) > cols, np.float32(-1e9), np.float32(0.0)
        )
    maskt = maskt.astype(ml_dtypes.bfloat16)
    idbf = np.zeros((128, 132), dtype=ml_dtypes.bfloat16)
    idbf[:, 0:128] = np.eye(128, dtype=ml_dtypes.bfloat16)
    idbf[:, 128:132] = ml_dtypes.bfloat16(1.0)

    consts = np.zeros((128, 193), dtype=np.float32)
    consts[:, 0:128] = np.eye(128, dtype=np.float32)
    consts[:, 128:193] = 1.0

    xt = {}
    for b in range(B):
        xt[("q", b)] = np.ascontiguousarray(query[b].T)
        xt[("k", b)] = np.ascontiguousarray(key[b].T)
        xt[("v", b)] = np.ascontiguousarray(value[b].T)

    in_maps = []
    for c in range(N_CORES):
        b = c // 4
        hb = (c % 4) * HPC
        rs = slice(hb * D_K, (hb + HPC) * D_K)
        in_maps.append(
            {
                "xq": xt[("q", b)],
                "xk": xt[("k", b)],
                "xv": xt[("v", b)],
                "wq": np.ascontiguousarray(w_q[rs, :].T),
                "wk": np.ascontiguousarray(w_k[rs, :].T),
                "wv": np.ascontiguousarray(w_v[rs, :].T),
                "wo": np.ascontiguousarray(w_o[:, rs].T),
                "maskt": maskt,
                "idbf": idbf,
                "consts": consts,
            }
        )
    return in_maps


def kernel(query, key, value, mask, w_q, w_k, w_v, w_o):
    from concourse.bass_utils import run_bass_kernel_spmd

    in_maps = _host_prep(query, key, value, mask, w_q, w_k, w_v, w_o)
    nc = _get_program()
    res = run_bass_kernel_spmd(nc, in_maps, list(range(N_CORES)))
    out = np.zeros((B, S, D_MODEL), dtype=np.float32)
    for c in range(N_CORES):
        out[c // 4] += res.results[c]["y"]
    return out



# revision 11
# speedup vs baseline: 1.0044x; 1.0044x over previous
"""Multi-head causal attention (B=2, S=2048, D=1024, H=16) on 8 TRN2 NeuronCores.

Sharding: batch*head parallel. Core c handles batch b = c//4 and the 4
heads h in [4*(c%4), 4*(c%4)+4). Each core computes its heads' Q/K/V
projections (column-parallel), causal softmax attention, and its partial
row-parallel output projection; the host sums the 4 partial outputs per
batch (the AllReduce of row-parallel tensor parallelism).

On-device layout: everything is kept "transposed" (feature-major) so
every matmul contracts along the partition dimension:
  scoresT[k,q] = K Q^T      (per head, 128-row k-tiles x 512-col q-tiles)
  P^T = exp(scoresT/8 + mask/8)   (additive -1e9 causal mask, PE-accumulated)
  outT[d,q]   = sum_k V[k,d] P^T[k,q]   (PSUM-accumulated over k-tiles)
  sums[q]     = sum_k P^T[k,q]          (ones-vector matmul, col-packed)
  y[q,e]     += sum_hd outT_norm[hd,q] * w_oT[hd,e]
Softmax skips the max-subtraction: scores ~ N(0,1), so exp never
overflows fp32, and exp(-1e9/8) underflows to exactly 0 like the
reference's masked_fill(-1e9).

Matmuls run as float32r (TF32-like, 1 cycle/row at N>=512; measured
~1.5e-4 rms per matmul). Fully-masked 128x512 blocks are skipped
(causal => ~62% of blocks computed).
"""

import numpy as np

D_MODEL = 1024
N_HEADS = 16
D_K = 64
B, S = 2, 2048
N_CORES = 8
HPC = 4            # heads per core
KT = S // 128      # 16 k-tiles
QT = S // 512      # 4 q-tiles
ET = D_MODEL // 128  # 8 e-tiles (contraction tiles for projections)

ATT_BF16 = False  # bf16 scores/attnV matmuls (f32r projections + output proj)
ET_BF16 = True   # bf16 exp output + V operand for the attnV matmul only

_PROG_CACHE = {}


def _build_program():
    import concourse.bacc as bacc_mod
    import concourse.mybir as mybir
    import concourse.tile as tile

    f32 = mybir.dt.float32
    f32r = mybir.dt.float32r
    bf16 = mybir.dt.bfloat16
    att_dt = bf16 if ATT_BF16 else f32r
    et_dt = bf16 if (ATT_BF16 or ET_BF16) else f32r
    Exp = mybir.ActivationFunctionType.Exp

    nc = bacc_mod.Bacc(
        "TRN2", target_bir_lowering=False, debug=False, num_devices=N_CORES
    )

    xq = nc.dram_tensor("xq", [D_MODEL, S], f32r, kind="ExternalInput").ap()
    xk = nc.dram_tensor("xk", [D_MODEL, S], f32r, kind="ExternalInput").ap()
    xv = nc.dram_tensor("xv", [D_MODEL, S], f32r, kind="ExternalInput").ap()
    wq = nc.dram_tensor("wq", [D_MODEL, 256], f32r, kind="ExternalInput").ap()
    wk = nc.dram_tensor("wk", [D_MODEL, 256], f32r, kind="ExternalInput").ap()
    wv = nc.dram_tensor("wv", [D_MODEL, 256], f32r, kind="ExternalInput").ap()
    wo = nc.dram_tensor("wo", [256, D_MODEL], f32r, kind="ExternalInput").ap()
    maskt = nc.dram_tensor("maskt", [128, 2048], mybir.dt.bfloat16, kind="ExternalInput").ap()
    idbf = nc.dram_tensor("idbf", [128, 132], mybir.dt.bfloat16, kind="ExternalInput").ap()
    consts = nc.dram_tensor("consts", [128, 193], f32r, kind="ExternalInput").ap()
    y = nc.dram_tensor("y", [S, D_MODEL], f32, kind="ExternalOutput").ap()

    with (
        tile.TileContext(nc) as tc,
        nc.allow_low_precision("fp32r attention"),
        tc.tile_pool(name="persist", bufs=1) as pp,
    ):
        # ---- persistent SBUF tiles ----
        def persist(shape, dtype, name):
            return pp.tile(shape, dtype, name=name, tag=name)

        wq_sb = persist([128, ET * 256], f32r, "wq_sb")
        wk_sb = persist([128, ET * 256], f32r, "wk_sb")
        wv_sb = persist([128, ET * 256], f32r, "wv_sb")
        wo_sb = [persist([128, D_MODEL], f32r, f"wo_sb{p}") for p in range(2)]
        maskt_sb = persist([128, 2048], mybir.dt.bfloat16, "maskt_sb")
        idbf_sb = persist([128, 132], mybir.dt.bfloat16, "idbf_sb")
        consts_sb = persist([128, 193], f32r, "consts_sb")
        qt_sb = [persist([128, S], att_dt, f"qt_sb{p}") for p in range(2)]
        kt_sb = [persist([128, S], att_dt, f"kt_sb{p}") for p in range(2)]
        v_sb = [persist([128, 260], et_dt, f"v_sb{i}") for i in range(KT)]
        outt_sb = [persist([128, S], f32r, f"outt_sb{p}") for p in range(2)]

        identity = consts_sb[:, 0:128]
        ones_col = consts_sb[:, 128:192]   # [128, 64] of 1.0
        ones1 = consts_sb[:, 192:193]      # [128, 1] of 1.0

        # small consts first (the PE warm-up pack depends on them); only wq up
        # front — the other weights are emitted mid-stream in phase B so the
        # first xq e-tile isn't stuck behind ~10us of serialized DMA issue.
        nc.sync.dma_start(out=consts_sb[:], in_=consts[:])
        nc.sync.dma_start(out=idbf_sb[:], in_=idbf[:])
        nc.sync.dma_start(out=maskt_sb[:], in_=maskt[:])

        def emit_w_dma(w_dram, w_tile):
            # weight load: [1024, 256] -> [128, 8*256] (e-tile t at cols 256t)
            nc.sync.dma_start(
                out=w_tile[:].rearrange("p (t d) -> p t d", t=ET),
                in_=w_dram.rearrange("(t p) d -> p t d", p=128),
            )

        emit_w_dma(wq, wq_sb)

        # ---- PE warm-up ----
        # The PE HAM clock gate starts (and re-enters) K=4/8 half-clock and
        # only returns to full clock after ~3.4us of gapless PE activity.
        # Dense same-stationary dummy matmuls (results never read) force the
        # transition; packs are re-issued wherever the schedule has an
        # unavoidable multi-us PE idle (DMA-bound ramp, phase boundaries,
        # softmax-normalize tails).
        def emit_warm_pack(pool, count, tag="warm", name="warm"):
            wt = pool.tile([128, 512], f32, name=name, tag=tag)
            for w in range(count):
                nc.tensor.matmul(
                    wt[:],
                    idbf_sb[:, 0:128],
                    maskt_sb[:, 0:512],
                    start=True,
                    stop=True,
                )

        with tc.tile_pool(name="psW", bufs=1, space="PSUM") as psW:
            emit_warm_pack(psW, 24, name="warm_start")

        # ---- phase B: projections ----
        # Q^T/K^T accumulate over all 8 e-tiles into [128, 2048] PSUM (8
        # banks, both m-tiles). The strided xv DMAs are emitted interleaved
        # with the xq/xk streams so the V-projection (which must wait for
        # the QK PSUM banks anyway) starts with its data already resident
        # and runs as a dense PE burst instead of being DMA-paced.
        with (
            tc.tile_pool(name="xe", bufs=3) as xep,
            tc.tile_pool(name="xvk", bufs=10) as xvkp,
        ):
            vdma_tiles = []

            def emit_v_dma():
                # issued from the gpsimd queue so the strided descriptor
                # programming doesn't serialize behind the xe stream on sync
                i = len(vdma_tiles)
                xvk = xvkp.tile([128, ET * 128], f32r, name=f"xvk_{i}", tag="xvk")
                nc.gpsimd.dma_start(
                    out=xvk[:].rearrange("p (t k) -> p t k", t=ET),
                    in_=xv[:, i * 128 : (i + 1) * 128].rearrange(
                        "(t p) k -> p t k", p=128
                    ),
                )
                vdma_tiles.append(xvk)

            psA_ctx = tc.tile_pool(name="psA", bufs=1, space="PSUM")
            psA = psA_ctx.__enter__()
            for ti, (x_dram, w_tile, dst) in enumerate(
                ((xq, wq_sb, qt_sb), (xk, wk_sb, kt_sb))
            ):
                ps = [
                    psA.tile(
                        [128, S], f32, name=f"ps_p{ti}_{m}", tag=f"proj{m}", bufs=1
                    )
                    for m in range(2)
                ]
                for e in range(ET):
                    xe = xep.tile([128, S], f32r, name=f"xe_{ti}_{e}", tag="xe")
                    nc.sync.dma_start(out=xe[:], in_=x_dram[e * 128 : (e + 1) * 128, :])
                    if ti == 0 and e == 0:
                        emit_w_dma(wk, wk_sb)
                    if ti == 0 and e == 4:
                        emit_w_dma(wv, wv_sb)
                    if ti == 1 and e == 0:
                        for p in range(2):
                            nc.sync.dma_start(
                                out=wo_sb[p][:], in_=wo[p * 128 : (p + 1) * 128, :]
                            )
                    if ti == 1 or e >= 6:
                        emit_v_dma()
                    for m in range(2):
                        lhsT = w_tile[:, e * 256 + m * 128 : e * 256 + (m + 1) * 128]
                        for n in range(QT):
                            nc.tensor.matmul(
                                ps[m][:, n * 512 : (n + 1) * 512],
                                lhsT,
                                xe[:, n * 512 : (n + 1) * 512],
                                start=(e == 0),
                                stop=(e == ET - 1),
                            )
                for m in range(2):
                    nc.vector.tensor_copy(dst[m][:], ps[m][:])

            psA_ctx.__exit__(None, None, None)
            psV_ctx = tc.tile_pool(name="psV", bufs=2, space="PSUM")
            psV = psV_ctx.__enter__()
            # V projection: dense burst (data already largely resident)
            for i in range(KT):
                if i >= len(vdma_tiles) - 2 and len(vdma_tiles) < KT:
                    emit_v_dma()
                psv = psV.tile([128, 256], f32, name=f"psv_{i}", tag="v")
                xvk = vdma_tiles[i]
                for e in range(ET):
                    nc.tensor.matmul(
                        psv[:],
                        xvk[:, e * 128 : (e + 1) * 128],
                        wv_sb[:, e * 256 : (e + 1) * 256],
                        start=(e == 0),
                        stop=(e == ET - 1),
                    )
                nc.vector.tensor_copy(
                    v_sb[i][:].rearrange("p (h c) -> p h c", c=65)[:, :, 0:64],
                    psv[:].rearrange("p (h d) -> p h d", d=64),
                )
                ones4 = idbf_sb[:, 128:132] if (ATT_BF16 or ET_BF16) else consts_sb[:, 128:132]
                nc.vector.tensor_copy(
                    v_sb[i][:].rearrange("p (h c) -> p h c", c=65)[:, :, 64:65],
                    ones4.rearrange("p (h c) -> p h c", c=1),
                )
            while len(vdma_tiles) < KT:
                emit_v_dma()
            psV_ctx.__exit__(None, None, None)

        # ---- phase C+D: attention with interleaved output projection ----
        # One head-pair per pass (pr = 0, 1). Per (pr, j): score tiles are
        # [128, 1024] head-pair PSUM tiles (row-packed score MMs fill the two
        # banks concurrently; ONE exp per round at FD=1024 runs ~2x faster
        # per element). attnV accumulates into a [65, 1024] pair tile (row
        # 64 = sum of exp via the ones column of v_sb). Normalization of
        # q-block j-1 is emitted lazily inside block j so its DVE chain and
        # broadcast matmuls never stall the PE; the output projection of
        # block j-1 runs as dense filler inside the pr=1 pass.
        with (
            tc.tile_pool(name="psS", bufs=2, space="PSUM") as psS,
            tc.tile_pool(name="psO", bufs=2, space="PSUM") as psO,
            tc.tile_pool(name="et", bufs=6) as etp,
            tc.tile_pool(name="bcsb", bufs=3) as bcp,
            tc.tile_pool(name="rcsb", bufs=3) as rcp,
            tc.tile_pool(name="ysb", bufs=3) as ysbp,
        ):
            def emit_outproj_mtile(m):
                psy = psS.tile([128, 1024], f32, name=f"psy_{m}", tag="s")
                for p in range(2):
                    for n in range(2):
                        nc.tensor.matmul(
                            psy[:, n * 512 : (n + 1) * 512],
                            outt_sb[p][:, m * 128 : (m + 1) * 128],
                            wo_sb[p][:, n * 512 : (n + 1) * 512],
                            start=(p == 0),
                            stop=(p == 1),
                        )
                y_sb = ysbp.tile([128, 1024], f32, name=f"y_sb_{m}", tag="ysb")
                nc.vector.tensor_copy(y_sb[:], psy[:])
                nc.sync.dma_start(out=y[m * 128 : (m + 1) * 128, :], in_=y_sb[:])

            USE_GPSIMD_NORM = False

            def emit_normalize(pr, jj, ps_out_prev):
                qsj = slice(jj * 512, (jj + 1) * 512)
                if USE_GPSIMD_NORM:
                    # reciprocal of the sums row straight out of PSUM, broadcast
                    # across 64 partitions on the (idle) GPSIMD engine: no PE
                    # involvement, so the in-order PE queue never stalls on this.
                    rc = rcp.tile([1, 1024], f32, name=f"rc_{pr}_{jj}", tag="rc")
                    nc.vector.reciprocal_approx_fast(
                        out=rc[:], in_=ps_out_prev[64:65, :]
                    )
                    bc_sb = bcp.tile([64, 1024], f32, name=f"bc_sb_{pr}_{jj}", tag="bc")
                    nc.gpsimd.partition_broadcast(bc_sb[:], rc[0:1, :])
                else:
                    ssb = rcp.tile([33, 512], f32, name=f"ssb_{pr}_{jj}", tag="ssb")
                    for hh in range(2):
                        nc.vector.tensor_copy(
                            ssb[32 * hh : 32 * hh + 1, :],
                            ps_out_prev[64:65, 512 * hh : 512 * (hh + 1)],
                        )
                    rc32 = rcp.tile([33, 512], f32, name=f"rc32_{pr}_{jj}", tag="rc32")
                    nc.vector.reciprocal_approx_fast(out=rc32[:], in_=ssb[:])
                    rc = rcp.tile([33, 512], f32r, name=f"rc_{pr}_{jj}", tag="rc")
                    nc.vector.tensor_copy(rc[:], rc32[:])
                    bc = psS.tile([128, 1024], f32, name=f"ps_bc_{pr}_{jj}", tag="s")
                    for hh in range(2):
                        nc.tensor.matmul(
                            bc[0:64, 512 * hh : 512 * (hh + 1)],
                            consts_sb[32 * hh : 32 * hh + 1, 128:192],
                            rc[32 * hh : 32 * hh + 1, :],
                            start=True,
                            stop=True,
                            tile_position=(32 * hh, 0),
                        )
                    bc_sb = bcp.tile([64, 1024], f32, name=f"bc_sb_{pr}_{jj}", tag="bc")
                    nc.vector.tensor_copy(bc_sb[:], bc[0:64, :])
                for hh in range(2):
                    nc.vector.tensor_mul(
                        outt_sb[pr][64 * hh : 64 * hh + 64, qsj],
                        ps_out_prev[0:64, 512 * hh : 512 * (hh + 1)],
                        bc_sb[:, 512 * hh : 512 * (hh + 1)],
                    )

            pending_norm = None  # (pr, j, ps_out) awaiting lazy normalize
            pending_out = []     # m-tiles awaiting output projection
            for pr in range(2):
                for j in range(QT):
                    n_i = 4 * j + 4
                    qs = slice(j * 512, (j + 1) * 512)
                    ps_out = psO.tile(
                        [65, 1024], f32, name=f"ps_out_{pr}_{j}", tag="o"
                    )
                    prev_et = None
                    prev_i = -1
                    for i in range(n_i):
                        diag = i >= 4 * j
                        r = i - 4 * j
                        pss = psS.tile(
                            [128, 1024], f32, name=f"ps_s{pr}_{j}_{i}", tag="s"
                        )
                        if diag:
                            nw = 128 * (r + 1)
                            for hh in range(2):
                                nc.tensor.matmul(
                                    pss[:, 512 * hh : 512 * hh + nw],
                                    idbf_sb[:, 0:128],
                                    maskt_sb[:, r * 512 : r * 512 + nw],
                                    start=True,
                                    stop=False,
                                )
                        for hh in range(2):
                            hp = slice(64 * hh, 64 * hh + 64)
                            nc.tensor.matmul(
                                pss[:, 512 * hh : 512 * (hh + 1)],
                                kt_sb[pr][hp, i * 128 : (i + 1) * 128],
                                qt_sb[pr][hp, qs],
                                start=not diag,
                                stop=True,
                            )
                        et = etp.tile(
                            [128, 1024], et_dt, name=f"et{pr}_{j}_{i}", tag="et"
                        )
                        nc.scalar.activation(et[:], pss[:], Exp, scale=0.125)
                        if prev_et is not None:
                            for hh in range(2):
                                nc.tensor.matmul(
                                    ps_out[:, 512 * hh : 512 * (hh + 1)],
                                    v_sb[prev_i][:, (2 * pr + hh) * 65 : (2 * pr + hh + 1) * 65],
                                    prev_et[:, 512 * hh : 512 * (hh + 1)],
                                    start=(prev_i == 0),
                                    stop=(prev_i == n_i - 1),
                                )
                        prev_et, prev_i = et, i
                        if i == 1 and pending_norm is not None:
                            pn_pr, pn_j = pending_norm[0], pending_norm[1]
                            emit_normalize(*pending_norm)
                            pending_norm = None
                            if pn_pr == 1:
                                pending_out.extend(range(4 * pn_j, 4 * pn_j + 4))
                        if pending_out and i >= 3:
                            emit_outproj_mtile(pending_out.pop(0))
                    for hh in range(2):
                        nc.tensor.matmul(
                            ps_out[:, 512 * hh : 512 * (hh + 1)],
                            v_sb[n_i - 1][:, (2 * pr + hh) * 65 : (2 * pr + hh + 1) * 65],
                            prev_et[:, 512 * hh : 512 * (hh + 1)],
                            start=(n_i - 1 == 0),
                            stop=True,
                        )
                    pending_norm = (pr, j, ps_out)
            # tail: last block's normalize + its output projection
            emit_normalize(*pending_norm)
            pending_out.extend(range(4 * (QT - 1), 4 * QT))
            for m in pending_out:
                emit_outproj_mtile(m)

    nc.compile()
    return nc


def _get_program():
    if "nc" not in _PROG_CACHE:
        _PROG_CACHE["nc"] = _build_program()
    return _PROG_CACHE["nc"]


def _host_prep(query, key, value, mask, w_q, w_k, w_v, w_o):
    query = np.asarray(query, dtype=np.float32)
    key = np.asarray(key, dtype=np.float32)
    value = np.asarray(value, dtype=np.float32)
    w_q = np.asarray(w_q, dtype=np.float32)
    w_k = np.asarray(w_k, dtype=np.float32)
    w_v = np.asarray(w_v, dtype=np.float32)
    w_o = np.asarray(w_o, dtype=np.float32)
    m = np.asarray(mask).reshape(S, S).astype(bool)

    # The kernel's block-skip structure assumes the standard causal mask.
    expected = np.triu(np.ones((S, S), dtype=bool), k=1)
    if not np.array_equal(m, expected):
        raise NotImplementedError("kernel specialized for causal (triu, k=1) mask")

    # 4 canonical diagonal-straddle mask tiles: pattern r covers k-tile
    # 4j+r vs q-tile j; masked where (128r + row) > col.
    import ml_dtypes

    maskt = np.zeros((128, 2048), dtype=np.float32)
    rows = np.arange(128)[:, None]
    cols = np.arange(512)[None, :]
    for r in range(4):
        maskt[:, r * 512 : (r + 1) * 512] = np.where(
            (128 * r + rows) > cols, np.float32(-1e9), np.float32(0.0)
        )
    maskt = maskt.astype(ml_dtypes.bfloat16)
    idbf = np.zeros((128, 132), dtype=ml_dtypes.bfloat16)
    idbf[:, 0:128] = np.eye(128, dtype=ml_dtypes.bfloat16)
    idbf[:, 128:132] = ml_dtypes.bfloat16(1.0)

    consts = np.zeros((128, 193), dtype=np.float32)
    consts[:, 0:128] = np.eye(128, dtype=np.float32)
    consts[:, 128:193] = 1.0

    xt = {}
    for b in range(B):
        xt[("q", b)] = np.ascontiguousarray(query[b].T)
        xt[("k", b)] = np.ascontiguousarray(key[b].T)
        xt[("v", b)] = np.ascontiguousarray(value[b].T)

    in_maps = []
    for c in range(N_CORES):
        b = c // 4
        hb = (c % 4) * HPC
        rs = slice(hb * D_K, (hb + HPC) * D_K)
        in_maps.append(
            {
                "xq": xt[("q", b)],
                "xk": xt[("k", b)],
                "xv": xt[("v", b)],
                "wq": np.ascontiguousarray(w_q[rs, :].T),
                "wk": np.ascontiguousarray(w_k[rs, :].T),
                "wv": np.ascontiguousarray(w_v[rs, :].T),
                "wo": np.ascontiguousarray(w_o[:, rs].T),
                "maskt": maskt,
                "idbf": idbf,
                "consts": consts,
            }
        )
    return in_maps


def kernel(query, key, value, mask, w_q, w_k, w_v, w_o):
    from concourse.bass_utils import run_bass_kernel_spmd

    in_maps = _host_prep(query, key, value, mask, w_q, w_k, w_v, w_o)
    nc = _get_program()
    res = run_bass_kernel_spmd(nc, in_maps, list(range(N_CORES)))
    out = np.zeros((B, S, D_MODEL), dtype=np.float32)
    for c in range(N_CORES):
        out[c // 4] += res.results[c]["y"]
    return out



# revision 13
# speedup vs baseline: 1.0249x; 1.0203x over previous
"""Multi-head causal attention (B=2, S=2048, D=1024, H=16) on 8 TRN2 NeuronCores.

Sharding: batch*head parallel. Core c handles batch b = c//4 and the 4
heads h in [4*(c%4), 4*(c%4)+4). Each core computes its heads' Q/K/V
projections (column-parallel), causal softmax attention, and its partial
row-parallel output projection; the host sums the 4 partial outputs per
batch (the AllReduce of row-parallel tensor parallelism).

On-device layout: everything is kept "transposed" (feature-major) so
every matmul contracts along the partition dimension:
  scoresT[k,q] = K Q^T      (per head, 128-row k-tiles x 512-col q-tiles)
  P^T = exp(scoresT/8 + mask/8)   (additive -1e9 causal mask, PE-accumulated)
  outT[d,q]   = sum_k V[k,d] P^T[k,q]   (PSUM-accumulated over k-tiles)
  sums[q]     = sum_k P^T[k,q]          (ones-vector matmul, col-packed)
  y[q,e]     += sum_hd outT_norm[hd,q] * w_oT[hd,e]
Softmax skips the max-subtraction: scores ~ N(0,1), so exp never
overflows fp32, and exp(-1e9/8) underflows to exactly 0 like the
reference's masked_fill(-1e9).

Matmuls run as float32r (TF32-like, 1 cycle/row at N>=512; measured
~1.5e-4 rms per matmul). Fully-masked 128x512 blocks are skipped
(causal => ~62% of blocks computed).
"""

import numpy as np

D_MODEL = 1024
N_HEADS = 16
D_K = 64
B, S = 2, 2048
N_CORES = 8
HPC = 4            # heads per core
KT = S // 128      # 16 k-tiles
QT = S // 512      # 4 q-tiles
ET = D_MODEL // 128  # 8 e-tiles (contraction tiles for projections)

ATT_BF16 = False  # bf16 scores/attnV matmuls (f32r projections + output proj)
ET_BF16 = True   # bf16 exp output + V operand for the attnV matmul only

_PROG_CACHE = {}


def _build_program():
    import concourse.bacc as bacc_mod
    import concourse.mybir as mybir
    import concourse.tile as tile

    f32 = mybir.dt.float32
    f32r = mybir.dt.float32r
    bf16 = mybir.dt.bfloat16
    att_dt = bf16 if ATT_BF16 else f32r
    et_dt = bf16 if (ATT_BF16 or ET_BF16) else f32r
    Exp = mybir.ActivationFunctionType.Exp

    nc = bacc_mod.Bacc(
        "TRN2", target_bir_lowering=False, debug=False, num_devices=N_CORES
    )

    xq = nc.dram_tensor("xq", [D_MODEL, S], f32r, kind="ExternalInput").ap()
    xk = nc.dram_tensor("xk", [D_MODEL, S], f32r, kind="ExternalInput").ap()
    xv = nc.dram_tensor("xv", [D_MODEL, S], f32r, kind="ExternalInput").ap()
    wq = nc.dram_tensor("wq", [D_MODEL, 256], f32r, kind="ExternalInput").ap()
    wk = nc.dram_tensor("wk", [D_MODEL, 256], f32r, kind="ExternalInput").ap()
    wv = nc.dram_tensor("wv", [D_MODEL, 256], f32r, kind="ExternalInput").ap()
    wo = nc.dram_tensor("wo", [256, D_MODEL], f32r, kind="ExternalInput").ap()
    maskt = nc.dram_tensor("maskt", [128, 2048], mybir.dt.bfloat16, kind="ExternalInput").ap()
    idbf = nc.dram_tensor("idbf", [128, 132], mybir.dt.bfloat16, kind="ExternalInput").ap()
    consts = nc.dram_tensor("consts", [128, 193], f32r, kind="ExternalInput").ap()
    y = nc.dram_tensor("y", [S, D_MODEL], f32, kind="ExternalOutput").ap()

    with (
        tile.TileContext(nc) as tc,
        nc.allow_low_precision("fp32r attention"),
        tc.tile_pool(name="persist", bufs=1) as pp,
    ):
        # ---- persistent SBUF tiles ----
        def persist(shape, dtype, name):
            return pp.tile(shape, dtype, name=name, tag=name)

        wq_sb = persist([128, ET * 256], f32r, "wq_sb")
        wk_sb = persist([128, ET * 256], f32r, "wk_sb")
        wv_sb = persist([128, ET * 256], f32r, "wv_sb")
        wo_sb = [persist([128, D_MODEL], f32r, f"wo_sb{p}") for p in range(2)]
        maskt_sb = persist([128, 2048], mybir.dt.bfloat16, "maskt_sb")
        idbf_sb = persist([128, 132], mybir.dt.bfloat16, "idbf_sb")
        consts_sb = persist([128, 193], f32r, "consts_sb")
        qt_sb = [persist([128, S], att_dt, f"qt_sb{p}") for p in range(2)]
        kt_sb = [persist([128, S], att_dt, f"kt_sb{p}") for p in range(2)]
        v_sb = [persist([128, 260], et_dt, f"v_sb{i}") for i in range(KT)]
        outt_sb = [persist([128, S], f32r, f"outt_sb{p}") for p in range(2)]

        identity = consts_sb[:, 0:128]
        ones_col = consts_sb[:, 128:192]   # [128, 64] of 1.0
        ones1 = consts_sb[:, 192:193]      # [128, 1] of 1.0

        # small consts first (the PE warm-up pack depends on them); only wq up
        # front — the other weights are emitted mid-stream in phase B so the
        # first xq e-tile isn't stuck behind ~10us of serialized DMA issue.
        nc.sync.dma_start(out=consts_sb[:], in_=consts[:])
        nc.sync.dma_start(out=idbf_sb[:], in_=idbf[:])
        nc.sync.dma_start(out=maskt_sb[:], in_=maskt[:])

        def emit_w_dma(w_dram, w_tile):
            # weight load: [1024, 256] -> [128, 8*256] (e-tile t at cols 256t)
            nc.sync.dma_start(
                out=w_tile[:].rearrange("p (t d) -> p t d", t=ET),
                in_=w_dram.rearrange("(t p) d -> p t d", p=128),
            )

        emit_w_dma(wq, wq_sb)

        # ---- PE warm-up ----
        # The PE HAM clock gate starts (and re-enters) K=4/8 half-clock and
        # only returns to full clock after ~3.4us of gapless PE activity.
        # Dense same-stationary dummy matmuls (results never read) force the
        # transition; packs are re-issued wherever the schedule has an
        # unavoidable multi-us PE idle (DMA-bound ramp, phase boundaries,
        # softmax-normalize tails).
        def emit_warm_pack(pool, count, tag="warm", name="warm"):
            wt = pool.tile([128, 512], f32, name=name, tag=tag)
            for w in range(count):
                nc.tensor.matmul(
                    wt[:],
                    idbf_sb[:, 0:128],
                    maskt_sb[:, 0:512],
                    start=True,
                    stop=True,
                )

        with tc.tile_pool(name="psW", bufs=1, space="PSUM") as psW:
            emit_warm_pack(psW, 24, name="warm_start")

        # ---- phase B: projections ----
        # Q^T/K^T accumulate over all 8 e-tiles into [128, 2048] PSUM (8
        # banks, both m-tiles). The strided xv DMAs are emitted interleaved
        # with the xq/xk streams so the V-projection (which must wait for
        # the QK PSUM banks anyway) starts with its data already resident
        # and runs as a dense PE burst instead of being DMA-paced.
        with (
            tc.tile_pool(name="xe", bufs=3) as xep,
            tc.tile_pool(name="xvk", bufs=10) as xvkp,
        ):
            vdma_tiles = []

            def emit_v_dma():
                # issued from the gpsimd queue so the strided descriptor
                # programming doesn't serialize behind the xe stream on sync
                i = len(vdma_tiles)
                xvk = xvkp.tile([128, ET * 128], f32r, name=f"xvk_{i}", tag="xvk")
                nc.gpsimd.dma_start(
                    out=xvk[:].rearrange("p (t k) -> p t k", t=ET),
                    in_=xv[:, i * 128 : (i + 1) * 128].rearrange(
                        "(t p) k -> p t k", p=128
                    ),
                )
                vdma_tiles.append(xvk)

            psA_ctx = tc.tile_pool(name="psA", bufs=1, space="PSUM")
            psA = psA_ctx.__enter__()
            for ti, (x_dram, w_tile, dst) in enumerate(
                ((xq, wq_sb, qt_sb), (xk, wk_sb, kt_sb))
            ):
                ps = [
                    psA.tile(
                        [128, S], f32, name=f"ps_p{ti}_{m}", tag=f"proj{m}", bufs=1
                    )
                    for m in range(2)
                ]
                for e in range(ET):
                    xe = xep.tile([128, S], f32r, name=f"xe_{ti}_{e}", tag="xe")
                    nc.sync.dma_start(out=xe[:], in_=x_dram[e * 128 : (e + 1) * 128, :])
                    if ti == 0 and e == 0:
                        emit_w_dma(wk, wk_sb)
                    if ti == 0 and e == 4:
                        emit_w_dma(wv, wv_sb)
                    if ti == 1 and e == 0:
                        for p in range(2):
                            nc.sync.dma_start(
                                out=wo_sb[p][:], in_=wo[p * 128 : (p + 1) * 128, :]
                            )
                    if ti == 1 or e >= 6:
                        emit_v_dma()
                    for m in range(2):
                        lhsT = w_tile[:, e * 256 + m * 128 : e * 256 + (m + 1) * 128]
                        for n in range(QT):
                            nc.tensor.matmul(
                                ps[m][:, n * 512 : (n + 1) * 512],
                                lhsT,
                                xe[:, n * 512 : (n + 1) * 512],
                                start=(e == 0),
                                stop=(e == ET - 1),
                            )
                for m in range(2):
                    nc.vector.tensor_copy(dst[m][:], ps[m][:])

            psA_ctx.__exit__(None, None, None)
            psV_ctx = tc.tile_pool(name="psV", bufs=2, space="PSUM")
            psV = psV_ctx.__enter__()
            # V projection: dense burst (data already largely resident)
            for i in range(KT):
                if i >= len(vdma_tiles) - 2 and len(vdma_tiles) < KT:
                    emit_v_dma()
                psv = psV.tile([128, 256], f32, name=f"psv_{i}", tag="v")
                xvk = vdma_tiles[i]
                for e in range(ET):
                    nc.tensor.matmul(
                        psv[:],
                        xvk[:, e * 128 : (e + 1) * 128],
                        wv_sb[:, e * 256 : (e + 1) * 256],
                        start=(e == 0),
                        stop=(e == ET - 1),
                    )
                nc.vector.tensor_copy(
                    v_sb[i][:].rearrange("p (h c) -> p h c", c=65)[:, :, 0:64],
                    psv[:].rearrange("p (h d) -> p h d", d=64),
                )
                ones4 = idbf_sb[:, 128:132] if (ATT_BF16 or ET_BF16) else consts_sb[:, 128:132]
                nc.vector.tensor_copy(
                    v_sb[i][:].rearrange("p (h c) -> p h c", c=65)[:, :, 64:65],
                    ones4.rearrange("p (h c) -> p h c", c=1),
                )
            while len(vdma_tiles) < KT:
                emit_v_dma()
            psV_ctx.__exit__(None, None, None)

        # ---- phase C+D: attention with interleaved output projection ----
        # One head-pair per pass (pr = 0, 1). Per (pr, j): score tiles are
        # [128, 1024] head-pair PSUM tiles (row-packed score MMs fill the two
        # banks concurrently; ONE exp per round at FD=1024 runs ~2x faster
        # per element). attnV accumulates into a [65, 1024] pair tile (row
        # 64 = sum of exp via the ones column of v_sb). Normalization of
        # q-block j-1 is emitted lazily inside block j so its DVE chain and
        # broadcast matmuls never stall the PE; the output projection of
        # block j-1 runs as dense filler inside the pr=1 pass.
        with (
            tc.tile_pool(name="psS", bufs=2, space="PSUM") as psS,
            tc.tile_pool(name="psO", bufs=2, space="PSUM") as psO,
            tc.tile_pool(name="et", bufs=6) as etp,
            tc.tile_pool(name="bcsb", bufs=3) as bcp,
            tc.tile_pool(name="rcsb", bufs=3) as rcp,
            tc.tile_pool(name="ysb", bufs=3) as ysbp,
        ):
            def emit_outproj_mtile(m):
                psy = psS.tile([128, 1024], f32, name=f"psy_{m}", tag="s")
                for p in range(2):
                    for n in range(2):
                        nc.tensor.matmul(
                            psy[:, n * 512 : (n + 1) * 512],
                            outt_sb[p][:, m * 128 : (m + 1) * 128],
                            wo_sb[p][:, n * 512 : (n + 1) * 512],
                            start=(p == 0),
                            stop=(p == 1),
                        )
                y_sb = ysbp.tile([128, 1024], f32, name=f"y_sb_{m}", tag="ysb")
                nc.vector.tensor_copy(y_sb[:], psy[:])
                nc.sync.dma_start(out=y[m * 128 : (m + 1) * 128, :], in_=y_sb[:])

            USE_GPSIMD_NORM = True

            def emit_normalize(pr, jj, ps_out_prev):
                qsj = slice(jj * 512, (jj + 1) * 512)
                if USE_GPSIMD_NORM:
                    # reciprocal of the sums row, broadcast across 64
                    # partitions on the (idle) GPSIMD engine: no PE
                    # involvement, so the in-order PE queue never stalls on
                    # this. The sums row is staged through SBUF (one copy per
                    # PSUM bank).
                    ssb = rcp.tile([1, 1024], f32, name=f"ssb_{pr}_{jj}", tag="ssb")
                    for hh in range(2):
                        nc.vector.tensor_copy(
                            ssb[0:1, 512 * hh : 512 * (hh + 1)],
                            ps_out_prev[64:65, 512 * hh : 512 * (hh + 1)],
                        )
                    rc = rcp.tile([1, 1024], f32, name=f"rc_{pr}_{jj}", tag="rc")
                    nc.vector.reciprocal_approx_fast(out=rc[:], in_=ssb[:])
                    bc_sb = bcp.tile([64, 1024], f32, name=f"bc_sb_{pr}_{jj}", tag="bc")
                    nc.gpsimd.partition_broadcast(bc_sb[:], rc[0:1, :])
                else:
                    ssb = rcp.tile([33, 512], f32, name=f"ssb_{pr}_{jj}", tag="ssb")
                    for hh in range(2):
                        nc.vector.tensor_copy(
                            ssb[32 * hh : 32 * hh + 1, :],
                            ps_out_prev[64:65, 512 * hh : 512 * (hh + 1)],
                        )
                    rc32 = rcp.tile([33, 512], f32, name=f"rc32_{pr}_{jj}", tag="rc32")
                    nc.vector.reciprocal_approx_fast(out=rc32[:], in_=ssb[:])
                    rc = rcp.tile([33, 512], f32r, name=f"rc_{pr}_{jj}", tag="rc")
                    nc.vector.tensor_copy(rc[:], rc32[:])
                    bc = psS.tile([128, 1024], f32, name=f"ps_bc_{pr}_{jj}", tag="s")
                    for hh in range(2):
                        nc.tensor.matmul(
                            bc[0:64, 512 * hh : 512 * (hh + 1)],
                            consts_sb[32 * hh : 32 * hh + 1, 128:192],
                            rc[32 * hh : 32 * hh + 1, :],
                            start=True,
                            stop=True,
                            tile_position=(32 * hh, 0),
                        )
                    bc_sb = bcp.tile([64, 1024], f32, name=f"bc_sb_{pr}_{jj}", tag="bc")
                    nc.vector.tensor_copy(bc_sb[:], bc[0:64, :])
                for hh in range(2):
                    nc.vector.tensor_mul(
                        outt_sb[pr][64 * hh : 64 * hh + 64, qsj],
                        ps_out_prev[0:64, 512 * hh : 512 * (hh + 1)],
                        bc_sb[:, 512 * hh : 512 * (hh + 1)],
                    )

            pending_norm = None  # (pr, j, ps_out) awaiting lazy normalize
            pending_out = []     # m-tiles awaiting output projection
            for pr in range(2):
                for j in range(QT):
                    n_i = 4 * j + 4
                    qs = slice(j * 512, (j + 1) * 512)
                    ps_out = psO.tile(
                        [65, 1024], f32, name=f"ps_out_{pr}_{j}", tag="o"
                    )
                    prev_et = None
                    prev_i = -1
                    for i in range(n_i):
                        diag = i >= 4 * j
                        r = i - 4 * j
                        pss = psS.tile(
                            [128, 1024], f32, name=f"ps_s{pr}_{j}_{i}", tag="s"
                        )
                        if diag:
                            nw = 128 * (r + 1)
                            for hh in range(2):
                                nc.tensor.matmul(
                                    pss[:, 512 * hh : 512 * hh + nw],
                                    idbf_sb[:, 0:128],
                                    maskt_sb[:, r * 512 : r * 512 + nw],
                                    start=True,
                                    stop=False,
                                )
                        for hh in range(2):
                            hp = slice(64 * hh, 64 * hh + 64)
                            nc.tensor.matmul(
                                pss[:, 512 * hh : 512 * (hh + 1)],
                                kt_sb[pr][hp, i * 128 : (i + 1) * 128],
                                qt_sb[pr][hp, qs],
                                start=not diag,
                                stop=True,
                            )
                        et = etp.tile(
                            [128, 1024], et_dt, name=f"et{pr}_{j}_{i}", tag="et"
                        )
                        nc.scalar.activation(et[:], pss[:], Exp, scale=0.125)
                        if prev_et is not None:
                            for hh in range(2):
                                nc.tensor.matmul(
                                    ps_out[:, 512 * hh : 512 * (hh + 1)],
                                    v_sb[prev_i][:, (2 * pr + hh) * 65 : (2 * pr + hh + 1) * 65],
                                    prev_et[:, 512 * hh : 512 * (hh + 1)],
                                    start=(prev_i == 0),
                                    stop=(prev_i == n_i - 1),
                                )
                        prev_et, prev_i = et, i
                        if i == 1 and pending_norm is not None:
                            pn_pr, pn_j = pending_norm[0], pending_norm[1]
                            emit_normalize(*pending_norm)
                            pending_norm = None
                            if pn_pr == 1:
                                pending_out.extend(range(4 * pn_j, 4 * pn_j + 4))
                        if pending_out and i >= 3:
                            emit_outproj_mtile(pending_out.pop(0))
                    for hh in range(2):
                        nc.tensor.matmul(
                            ps_out[:, 512 * hh : 512 * (hh + 1)],
                            v_sb[n_i - 1][:, (2 * pr + hh) * 65 : (2 * pr + hh + 1) * 65],
                            prev_et[:, 512 * hh : 512 * (hh + 1)],
                            start=(n_i - 1 == 0),
                            stop=True,
                        )
                    pending_norm = (pr, j, ps_out)
            # tail: last block's normalize + its output projection
            emit_normalize(*pending_norm)
            pending_out.extend(range(4 * (QT - 1), 4 * QT))
            for m in pending_out:
                emit_outproj_mtile(m)

    nc.compile()
    return nc


def _get_program():
    if "nc" not in _PROG_CACHE:
        _PROG_CACHE["nc"] = _build_program()
    return _PROG_CACHE["nc"]


def _host_prep(query, key, value, mask, w_q, w_k, w_v, w_o):
    query = np.asarray(query, dtype=np.float32)
    key = np.asarray(key, dtype=np.float32)
    value = np.asarray(value, dtype=np.float32)
    w_q = np.asarray(w_q, dtype=np.float32)
    w_k = np.asarray(w_k, dtype=np.float32)
    w_v = np.asarray(w_v, dtype=np.float32)
    w_o = np.asarray(w_o, dtype=np.float32)
    m = np.asarray(mask).reshape(S, S).astype(bool)

    # The kernel's block-skip structure assumes the standard causal mask.
    expected = np.triu(np.ones((S, S), dtype=bool), k=1)
    if not np.array_equal(m, expected):
        raise NotImplementedError("kernel specialized for causal (triu, k=1) mask")

    # 4 canonical diagonal-straddle mask tiles: pattern r covers k-tile
    # 4j+r vs q-tile j; masked where (128r + row) > col.
    import ml_dtypes

    maskt = np.zeros((128, 2048), dtype=np.float32)
    rows = np.arange(128)[:, None]
    cols = np.arange(512)[None, :]
    for r in range(4):
        maskt[:, r * 512 : (r + 1) * 512] = np.where(
            (128 * r + rows) > cols, np.float32(-1e9), np.float32(0.0)
        )
    maskt = maskt.astype(ml_dtypes.bfloat16)
    idbf = np.zeros((128, 132), dtype=ml_dtypes.bfloat16)
    idbf[:, 0:128] = np.eye(128, dtype=ml_dtypes.bfloat16)
    idbf[:, 128:132] = ml_dtypes.bfloat16(1.0)

    consts = np.zeros((128, 193), dtype=np.float32)
    consts[:, 0:128] = np.eye(128, dtype=np.float32)
    consts[:, 128:193] = 1.0

    xt = {}
    for b in range(B):
        xt[("q", b)] = np.ascontiguousarray(query[b].T)
        xt[("k", b)] = np.ascontiguousarray(key[b].T)
        xt[("v", b)] = np.ascontiguousarray(value[b].T)

    in_maps = []
    for c in range(N_CORES):
        b = c // 4
        hb = (c % 4) * HPC
        rs = slice(hb * D_K, (hb + HPC) * D_K)
        in_maps.append(
            {
                "xq": xt[("q", b)],
                "xk": xt[("k", b)],
                "xv": xt[("v", b)],
                "wq": np.ascontiguousarray(w_q[rs, :].T),
                "wk": np.ascontiguousarray(w_k[rs, :].T),
                "wv": np.ascontiguousarray(w_v[rs, :].T),
                "wo": np.ascontiguousarray(w_o[:, rs].T),
                "maskt": maskt,
                "idbf": idbf,
                "consts": consts,
            }
        )
    return in_maps


def kernel(query, key, value, mask, w_q, w_k, w_v, w_o):
    from concourse.bass_utils import run_bass_kernel_spmd

    in_maps = _host_prep(query, key, value, mask, w_q, w_k, w_v, w_o)
    nc = _get_program()
    res = run_bass_kernel_spmd(nc, in_maps, list(range(N_CORES)))
    out = np.zeros((B, S, D_MODEL), dtype=np.float32)
    for c in range(N_CORES):
        out[c // 4] += res.results[c]["y"]
    return out



# revision 14
# speedup vs baseline: 1.0913x; 1.0648x over previous
"""Multi-head causal attention (B=2, S=2048, D=1024, H=16) on 8 TRN2 NeuronCores.

Sharding: batch*head parallel. Core c handles batch b = c//4 and the 4
heads h in [4*(c%4), 4*(c%4)+4). Each core computes its heads' Q/K/V
projections (column-parallel), causal softmax attention, and its partial
row-parallel output projection; the host sums the 4 partial outputs per
batch (the AllReduce of row-parallel tensor parallelism).

On-device layout: everything is kept "transposed" (feature-major) so
every matmul contracts along the partition dimension:
  scoresT[k,q] = K Q^T      (per head, 128-row k-tiles x 512-col q-tiles)
  P^T = exp(scoresT/8 + mask/8)   (additive -1e9 causal mask, PE-accumulated)
  outT[d,q]   = sum_k V[k,d] P^T[k,q]   (PSUM-accumulated over k-tiles)
  sums[q]     = sum_k P^T[k,q]          (ones-vector matmul, col-packed)
  y[q,e]     += sum_hd outT_norm[hd,q] * w_oT[hd,e]
Softmax skips the max-subtraction: scores ~ N(0,1), so exp never
overflows fp32, and exp(-1e9/8) underflows to exactly 0 like the
reference's masked_fill(-1e9).

Matmuls run as float32r (TF32-like, 1 cycle/row at N>=512; measured
~1.5e-4 rms per matmul). Fully-masked 128x512 blocks are skipped
(causal => ~62% of blocks computed).
"""

import numpy as np

D_MODEL = 1024
N_HEADS = 16
D_K = 64
B, S = 2, 2048
N_CORES = 8
HPC = 4            # heads per core
KT = S // 128      # 16 k-tiles
QT = S // 512      # 4 q-tiles
ET = D_MODEL // 128  # 8 e-tiles (contraction tiles for projections)

ATT_BF16 = True  # bf16 scores/attnV matmuls (f32r projections + output proj)
ET_BF16 = True   # bf16 exp output + V operand for the attnV matmul only

_PROG_CACHE = {}


def _build_program():
    import concourse.bacc as bacc_mod
    import concourse.mybir as mybir
    import concourse.tile as tile

    f32 = mybir.dt.float32
    f32r = mybir.dt.float32r
    bf16 = mybir.dt.bfloat16
    att_dt = bf16 if ATT_BF16 else f32r
    et_dt = bf16 if (ATT_BF16 or ET_BF16) else f32r
    Exp = mybir.ActivationFunctionType.Exp

    nc = bacc_mod.Bacc(
        "TRN2", target_bir_lowering=False, debug=False, num_devices=N_CORES
    )

    xq = nc.dram_tensor("xq", [D_MODEL, S], f32r, kind="ExternalInput").ap()
    xk = nc.dram_tensor("xk", [D_MODEL, S], f32r, kind="ExternalInput").ap()
    xv = nc.dram_tensor("xv", [D_MODEL, S], f32r, kind="ExternalInput").ap()
    wq = nc.dram_tensor("wq", [D_MODEL, 256], f32r, kind="ExternalInput").ap()
    wk = nc.dram_tensor("wk", [D_MODEL, 256], f32r, kind="ExternalInput").ap()
    wv = nc.dram_tensor("wv", [D_MODEL, 256], f32r, kind="ExternalInput").ap()
    wo = nc.dram_tensor("wo", [256, D_MODEL], f32r, kind="ExternalInput").ap()
    maskt = nc.dram_tensor("maskt", [128, 2048], mybir.dt.bfloat16, kind="ExternalInput").ap()
    idbf = nc.dram_tensor("idbf", [128, 132], mybir.dt.bfloat16, kind="ExternalInput").ap()
    consts = nc.dram_tensor("consts", [128, 193], f32r, kind="ExternalInput").ap()
    y = nc.dram_tensor("y", [S, D_MODEL], f32, kind="ExternalOutput").ap()

    with (
        tile.TileContext(nc) as tc,
        nc.allow_low_precision("fp32r attention"),
        tc.tile_pool(name="persist", bufs=1) as pp,
    ):
        # ---- persistent SBUF tiles ----
        def persist(shape, dtype, name):
            return pp.tile(shape, dtype, name=name, tag=name)

        wq_sb = persist([128, ET * 256], f32r, "wq_sb")
        wk_sb = persist([128, ET * 256], f32r, "wk_sb")
        wv_sb = persist([128, ET * 256], f32r, "wv_sb")
        wo_sb = [persist([128, D_MODEL], f32r, f"wo_sb{p}") for p in range(2)]
        maskt_sb = persist([128, 2048], mybir.dt.bfloat16, "maskt_sb")
        idbf_sb = persist([128, 132], mybir.dt.bfloat16, "idbf_sb")
        consts_sb = persist([128, 193], f32r, "consts_sb")
        qt_sb = [persist([128, S], att_dt, f"qt_sb{p}") for p in range(2)]
        kt_sb = [persist([128, S], att_dt, f"kt_sb{p}") for p in range(2)]
        v_sb = [persist([128, 260], et_dt, f"v_sb{i}") for i in range(KT)]
        outt_sb = [persist([128, S], f32r, f"outt_sb{p}") for p in range(2)]

        identity = consts_sb[:, 0:128]
        ones_col = consts_sb[:, 128:192]   # [128, 64] of 1.0
        ones1 = consts_sb[:, 192:193]      # [128, 1] of 1.0

        # small consts first (the PE warm-up pack depends on them); only wq up
        # front — the other weights are emitted mid-stream in phase B so the
        # first xq e-tile isn't stuck behind ~10us of serialized DMA issue.
        nc.sync.dma_start(out=consts_sb[:], in_=consts[:])
        nc.sync.dma_start(out=idbf_sb[:], in_=idbf[:])
        nc.sync.dma_start(out=maskt_sb[:], in_=maskt[:])

        def emit_w_dma(w_dram, w_tile):
            # weight load: [1024, 256] -> [128, 8*256] (e-tile t at cols 256t)
            nc.sync.dma_start(
                out=w_tile[:].rearrange("p (t d) -> p t d", t=ET),
                in_=w_dram.rearrange("(t p) d -> p t d", p=128),
            )

        emit_w_dma(wq, wq_sb)

        # ---- PE warm-up ----
        # The PE HAM clock gate starts (and re-enters) K=4/8 half-clock and
        # only returns to full clock after ~3.4us of gapless PE activity.
        # Dense same-stationary dummy matmuls (results never read) force the
        # transition; packs are re-issued wherever the schedule has an
        # unavoidable multi-us PE idle (DMA-bound ramp, phase boundaries,
        # softmax-normalize tails).
        def emit_warm_pack(pool, count, tag="warm", name="warm"):
            wt = pool.tile([128, 512], f32, name=name, tag=tag)
            for w in range(count):
                nc.tensor.matmul(
                    wt[:],
                    idbf_sb[:, 0:128],
                    maskt_sb[:, 0:512],
                    start=True,
                    stop=True,
                )

        with tc.tile_pool(name="psW", bufs=1, space="PSUM") as psW:
            emit_warm_pack(psW, 24, name="warm_start")

        # ---- phase B: projections ----
        # Q^T/K^T accumulate over all 8 e-tiles into [128, 2048] PSUM (8
        # banks, both m-tiles). The strided xv DMAs are emitted interleaved
        # with the xq/xk streams so the V-projection (which must wait for
        # the QK PSUM banks anyway) starts with its data already resident
        # and runs as a dense PE burst instead of being DMA-paced.
        with (
            tc.tile_pool(name="xe", bufs=3) as xep,
            tc.tile_pool(name="xvk", bufs=10) as xvkp,
        ):
            vdma_tiles = []

            def emit_v_dma():
                # issued from the gpsimd queue so the strided descriptor
                # programming doesn't serialize behind the xe stream on sync
                i = len(vdma_tiles)
                xvk = xvkp.tile([128, ET * 128], f32r, name=f"xvk_{i}", tag="xvk")
                nc.gpsimd.dma_start(
                    out=xvk[:].rearrange("p (t k) -> p t k", t=ET),
                    in_=xv[:, i * 128 : (i + 1) * 128].rearrange(
                        "(t p) k -> p t k", p=128
                    ),
                )
                vdma_tiles.append(xvk)

            psA_ctx = tc.tile_pool(name="psA", bufs=1, space="PSUM")
            psA = psA_ctx.__enter__()
            for ti, (x_dram, w_tile, dst) in enumerate(
                ((xq, wq_sb, qt_sb), (xk, wk_sb, kt_sb))
            ):
                ps = [
                    psA.tile(
                        [128, S], f32, name=f"ps_p{ti}_{m}", tag=f"proj{m}", bufs=1
                    )
                    for m in range(2)
                ]
                for e in range(ET):
                    xe = xep.tile([128, S], f32r, name=f"xe_{ti}_{e}", tag="xe")
                    nc.sync.dma_start(out=xe[:], in_=x_dram[e * 128 : (e + 1) * 128, :])
                    if ti == 0 and e == 0:
                        emit_w_dma(wk, wk_sb)
                    if ti == 0 and e == 4:
                        emit_w_dma(wv, wv_sb)
                    if ti == 1 and e == 0:
                        for p in range(2):
                            nc.sync.dma_start(
                                out=wo_sb[p][:], in_=wo[p * 128 : (p + 1) * 128, :]
                            )
                    if ti == 1 or e >= 6:
                        emit_v_dma()
                    for m in range(2):
                        lhsT = w_tile[:, e * 256 + m * 128 : e * 256 + (m + 1) * 128]
                        for n in range(QT):
                            nc.tensor.matmul(
                                ps[m][:, n * 512 : (n + 1) * 512],
                                lhsT,
                                xe[:, n * 512 : (n + 1) * 512],
                                start=(e == 0),
                                stop=(e == ET - 1),
                            )
                for m in range(2):
                    nc.vector.tensor_copy(dst[m][:], ps[m][:])

            psA_ctx.__exit__(None, None, None)
            psV_ctx = tc.tile_pool(name="psV", bufs=2, space="PSUM")
            psV = psV_ctx.__enter__()
            # V projection: dense burst (data already largely resident)
            for i in range(KT):
                if i >= len(vdma_tiles) - 2 and len(vdma_tiles) < KT:
                    emit_v_dma()
                psv = psV.tile([128, 256], f32, name=f"psv_{i}", tag="v")
                xvk = vdma_tiles[i]
                for e in range(ET):
                    nc.tensor.matmul(
                        psv[:],
                        xvk[:, e * 128 : (e + 1) * 128],
                        wv_sb[:, e * 256 : (e + 1) * 256],
                        start=(e == 0),
                        stop=(e == ET - 1),
                    )
                nc.vector.tensor_copy(
                    v_sb[i][:].rearrange("p (h c) -> p h c", c=65)[:, :, 0:64],
                    psv[:].rearrange("p (h d) -> p h d", d=64),
                )
                ones4 = idbf_sb[:, 128:132] if (ATT_BF16 or ET_BF16) else consts_sb[:, 128:132]
                nc.vector.tensor_copy(
                    v_sb[i][:].rearrange("p (h c) -> p h c", c=65)[:, :, 64:65],
                    ones4.rearrange("p (h c) -> p h c", c=1),
                )
            while len(vdma_tiles) < KT:
                emit_v_dma()
            psV_ctx.__exit__(None, None, None)

        # ---- phase C+D: attention with interleaved output projection ----
        # One head-pair per pass (pr = 0, 1). Per (pr, j): score tiles are
        # [128, 1024] head-pair PSUM tiles (row-packed score MMs fill the two
        # banks concurrently; ONE exp per round at FD=1024 runs ~2x faster
        # per element). attnV accumulates into a [65, 1024] pair tile (row
        # 64 = sum of exp via the ones column of v_sb). Normalization of
        # q-block j-1 is emitted lazily inside block j so its DVE chain and
        # broadcast matmuls never stall the PE; the output projection of
        # block j-1 runs as dense filler inside the pr=1 pass.
        with (
            tc.tile_pool(name="psS", bufs=2, space="PSUM") as psS,
            tc.tile_pool(name="psO", bufs=2, space="PSUM") as psO,
            tc.tile_pool(name="et", bufs=6) as etp,
            tc.tile_pool(name="bcsb", bufs=3) as bcp,
            tc.tile_pool(name="rcsb", bufs=3) as rcp,
            tc.tile_pool(name="ysb", bufs=3) as ysbp,
        ):
            def emit_outproj_mtile(m):
                psy = psS.tile([128, 1024], f32, name=f"psy_{m}", tag="s")
                for p in range(2):
                    for n in range(2):
                        nc.tensor.matmul(
                            psy[:, n * 512 : (n + 1) * 512],
                            outt_sb[p][:, m * 128 : (m + 1) * 128],
                            wo_sb[p][:, n * 512 : (n + 1) * 512],
                            start=(p == 0),
                            stop=(p == 1),
                        )
                y_sb = ysbp.tile([128, 1024], f32, name=f"y_sb_{m}", tag="ysb")
                nc.vector.tensor_copy(y_sb[:], psy[:])
                nc.sync.dma_start(out=y[m * 128 : (m + 1) * 128, :], in_=y_sb[:])

            USE_GPSIMD_NORM = True

            def emit_normalize(pr, jj, ps_out_prev):
                qsj = slice(jj * 512, (jj + 1) * 512)
                if USE_GPSIMD_NORM:
                    # reciprocal of the sums row, broadcast across 64
                    # partitions on the (idle) GPSIMD engine: no PE
                    # involvement, so the in-order PE queue never stalls on
                    # this. The sums row is staged through SBUF (one copy per
                    # PSUM bank).
                    ssb = rcp.tile([1, 1024], f32, name=f"ssb_{pr}_{jj}", tag="ssb")
                    for hh in range(2):
                        nc.vector.tensor_copy(
                            ssb[0:1, 512 * hh : 512 * (hh + 1)],
                            ps_out_prev[64:65, 512 * hh : 512 * (hh + 1)],
                        )
                    rc = rcp.tile([1, 1024], f32, name=f"rc_{pr}_{jj}", tag="rc")
                    nc.vector.reciprocal_approx_fast(out=rc[:], in_=ssb[:])
                    bc_sb = bcp.tile([64, 1024], f32, name=f"bc_sb_{pr}_{jj}", tag="bc")
                    nc.gpsimd.partition_broadcast(bc_sb[:], rc[0:1, :])
                else:
                    ssb = rcp.tile([33, 512], f32, name=f"ssb_{pr}_{jj}", tag="ssb")
                    for hh in range(2):
                        nc.vector.tensor_copy(
                            ssb[32 * hh : 32 * hh + 1, :],
                            ps_out_prev[64:65, 512 * hh : 512 * (hh + 1)],
                        )
                    rc32 = rcp.tile([33, 512], f32, name=f"rc32_{pr}_{jj}", tag="rc32")
                    nc.vector.reciprocal_approx_fast(out=rc32[:], in_=ssb[:])
                    rc = rcp.tile([33, 512], f32r, name=f"rc_{pr}_{jj}", tag="rc")
                    nc.vector.tensor_copy(rc[:], rc32[:])
                    bc = psS.tile([128, 1024], f32, name=f"ps_bc_{pr}_{jj}", tag="s")
                    for hh in range(2):
                        nc.tensor.matmul(
                            bc[0:64, 512 * hh : 512 * (hh + 1)],
                            consts_sb[32 * hh : 32 * hh + 1, 128:192],
                            rc[32 * hh : 32 * hh + 1, :],
                            start=True,
                            stop=True,
                            tile_position=(32 * hh, 0),
                        )
                    bc_sb = bcp.tile([64, 1024], f32, name=f"bc_sb_{pr}_{jj}", tag="bc")
                    nc.vector.tensor_copy(bc_sb[:], bc[0:64, :])
                for hh in range(2):
                    nc.vector.tensor_mul(
                        outt_sb[pr][64 * hh : 64 * hh + 64, qsj],
                        ps_out_prev[0:64, 512 * hh : 512 * (hh + 1)],
                        bc_sb[:, 512 * hh : 512 * (hh + 1)],
                    )

            pending_norm = None  # (pr, j, ps_out) awaiting lazy normalize
            pending_out = []     # m-tiles awaiting output projection
            for pr in range(2):
                for j in range(QT):
                    n_i = 4 * j + 4
                    qs = slice(j * 512, (j + 1) * 512)
                    ps_out = psO.tile(
                        [65, 1024], f32, name=f"ps_out_{pr}_{j}", tag="o"
                    )
                    prev_et = None
                    prev_i = -1
                    for i in range(n_i):
                        diag = i >= 4 * j
                        r = i - 4 * j
                        pss = psS.tile(
                            [128, 1024], f32, name=f"ps_s{pr}_{j}_{i}", tag="s"
                        )
                        if diag:
                            nw = 128 * (r + 1)
                            for hh in range(2):
                                nc.tensor.matmul(
                                    pss[:, 512 * hh : 512 * hh + nw],
                                    idbf_sb[:, 0:128],
                                    maskt_sb[:, r * 512 : r * 512 + nw],
                                    start=True,
                                    stop=False,
                                )
                        for hh in range(2):
                            hp = slice(64 * hh, 64 * hh + 64)
                            nc.tensor.matmul(
                                pss[:, 512 * hh : 512 * (hh + 1)],
                                kt_sb[pr][hp, i * 128 : (i + 1) * 128],
                                qt_sb[pr][hp, qs],
                                start=not diag,
                                stop=True,
                            )
                        et = etp.tile(
                            [128, 1024], et_dt, name=f"et{pr}_{j}_{i}", tag="et"
                        )
                        nc.scalar.activation(et[:], pss[:], Exp, scale=0.125)
                        if prev_et is not None:
                            for hh in range(2):
                                nc.tensor.matmul(
                                    ps_out[:, 512 * hh : 512 * (hh + 1)],
                                    v_sb[prev_i][:, (2 * pr + hh) * 65 : (2 * pr + hh + 1) * 65],
                                    prev_et[:, 512 * hh : 512 * (hh + 1)],
                                    start=(prev_i == 0),
                                    stop=(prev_i == n_i - 1),
                                )
                        prev_et, prev_i = et, i
                        if i == 1 and pending_norm is not None:
                            pn_pr, pn_j = pending_norm[0], pending_norm[1]
                            emit_normalize(*pending_norm)
                            pending_norm = None
                            if pn_pr == 1:
                                pending_out.extend(range(4 * pn_j, 4 * pn_j + 4))
                        if pending_out and i >= 3:
                            emit_outproj_mtile(pending_out.pop(0))
                    for hh in range(2):
                        nc.tensor.matmul(
                            ps_out[:, 512 * hh : 512 * (hh + 1)],
                            v_sb[n_i - 1][:, (2 * pr + hh) * 65 : (2 * pr + hh + 1) * 65],
                            prev_et[:, 512 * hh : 512 * (hh + 1)],
                            start=(n_i - 1 == 0),
                            stop=True,
                        )
                    pending_norm = (pr, j, ps_out)
            # tail: last block's normalize + its output projection
            emit_normalize(*pending_norm)
            pending_out.extend(range(4 * (QT - 1), 4 * QT))
            for m in pending_out:
                emit_outproj_mtile(m)

    nc.compile()
    return nc


def _get_program():
    if "nc" not in _PROG_CACHE:
        _PROG_CACHE["nc"] = _build_program()
    return _PROG_CACHE["nc"]


def _host_prep(query, key, value, mask, w_q, w_k, w_v, w_o):
    query = np.asarray(query, dtype=np.float32)
    key = np.asarray(key, dtype=np.float32)
    value = np.asarray(value, dtype=np.float32)
    w_q = np.asarray(w_q, dtype=np.float32)
    w_k = np.asarray(w_k, dtype=np.float32)
    w_v = np.asarray(w_v, dtype=np.float32)
    w_o = np.asarray(w_o, dtype=np.float32)
    m = np.asarray(mask).reshape(S, S).astype(bool)

    # The kernel's block-skip structure assumes the standard causal mask.
    expected = np.triu(np.ones((S, S), dtype=bool), k=1)
    if not np.array_equal(m, expected):
        raise NotImplementedError("kernel specialized for causal (triu, k=1) mask")

    # 4 canonical diagonal-straddle mask tiles: pattern r covers k-tile
    # 4j+r vs q-tile j; masked where (128r + row) > col.
    import ml_dtypes

    maskt = np.zeros((128, 2048), dtype=np.float32)
    rows = np.arange(128)[:, None]
    cols = np.arange(512)[None, :]
    for r in range(4):
        maskt[:, r * 512 : (r + 1) * 512] = np.where(
            (128 * r + rows) > cols, np.float32(-1e9), np.float32(0.0)
        )
    maskt = maskt.astype(ml_dtypes.bfloat16)
    idbf = np.zeros((128, 132), dtype=ml_dtypes.bfloat16)
    idbf[:, 0:128] = np.eye(128, dtype=ml_dtypes.bfloat16)
    idbf[:, 128:132] = ml_dtypes.bfloat16(1.0)

    consts = np.zeros((128, 193), dtype=np.float32)
    consts[:, 0:128] = np.eye(128, dtype=np.float32)
    consts[:, 128:193] = 1.0

    xt = {}
    for b in range(B):
        xt[("q", b)] = np.ascontiguousarray(query[b].T)
        xt[("k", b)] = np.ascontiguousarray(key[b].T)
        xt[("v", b)] = np.ascontiguousarray(value[b].T)

    in_maps = []
    for c in range(N_CORES):
        b = c // 4
        hb = (c % 4) * HPC
        rs = slice(hb * D_K, (hb + HPC) * D_K)
        in_maps.append(
            {
                "xq": xt[("q", b)],
                "xk": xt[("k", b)],
                "xv": xt[("v", b)],
                "wq": np.ascontiguousarray(w_q[rs, :].T),
                "wk": np.ascontiguousarray(w_k[rs, :].T),
                "wv": np.ascontiguousarray(w_v[rs, :].T),
                "wo": np.ascontiguousarray(w_o[:, rs].T),
                "maskt": maskt,
                "idbf": idbf,
                "consts": consts,
            }
        )
    return in_maps


def kernel(query, key, value, mask, w_q, w_k, w_v, w_o):
    from concourse.bass_utils import run_bass_kernel_spmd

    in_maps = _host_prep(query, key, value, mask, w_q, w_k, w_v, w_o)
    nc = _get_program()
    res = run_bass_kernel_spmd(nc, in_maps, list(range(N_CORES)))
    out = np.zeros((B, S, D_MODEL), dtype=np.float32)
    for c in range(N_CORES):
        out[c // 4] += res.results[c]["y"]
    return out



# revision 17
# speedup vs baseline: 1.2373x; 1.1338x over previous
"""Multi-head causal attention (B=2, S=2048, D=1024, H=16) on 8 TRN2 NeuronCores.

Sharding: batch*head parallel. Core c handles batch b = c//4 and the 4
heads h in [4*(c%4), 4*(c%4)+4). Each core computes its heads' Q/K/V
projections (column-parallel), causal softmax attention, and its partial
row-parallel output projection; the host sums the 4 partial outputs per
batch (the AllReduce of row-parallel tensor parallelism).

Streamed-chunk schedule: x is brought in as 512-column chunks (q/k/v per
round) and projected in PE bursts between attention blocks, so the
DMA-bound projection phase hides entirely under the PE/exp-bound
attention stream and the PE never idles long enough to drop the HAM
clock gate to half speed. Per round c: attention(pr0,c) -> output
projection of block c-1 -> attention(pr1,c) -> projection of chunk c+1
-> DMA issue for chunk c+2.

On-device layout: everything is kept "transposed" (feature-major) so
every matmul contracts along the partition dimension:
  scoresT[k,q] = K Q^T      (per head, 128-row k-tiles x 512-col q-tiles)
  P^T = exp(scoresT/8 + mask/8)   (additive -1e9 causal mask, PE-accumulated)
  outT[d,q]   = sum_k V[k,d] P^T[k,q]   (PSUM-accumulated over k-tiles)
  sums[q]     = sum_k P^T[k,q]          (ones-vector matmul, col-packed)
  y[q,e]     += sum_hd outT_norm[hd,q] * w_oT[hd,e]
Softmax skips the max-subtraction: scores ~ N(0,1), so exp never
overflows fp32, and exp(-1e9/8) underflows to exactly 0 like the
reference's masked_fill(-1e9).

Projections run as float32r (TF32-like); scores/attnV/output-projection
run bf16 (measured ~30% faster per moving row on this part). The softmax
normalize runs entirely off the PE: reciprocal on DVE, partition
broadcast on GPSIMD, so the in-order PE queue never stalls on it.
Fully-masked 128x512 blocks are skipped (causal => ~62% computed).
"""

import numpy as np

D_MODEL = 1024
N_HEADS = 16
D_K = 64
B, S = 2, 2048
N_CORES = 8
HPC = 4              # heads per core
KT = S // 128        # 16 k-tiles
QT = S // 512        # 4 q-tiles == x chunks
ET = D_MODEL // 128  # 8 e-tiles (contraction tiles for projections)

WARM0 = 10           # initial PE warm-up matmuls (cover the first DMAs)
WARM_TAIL = 6        # PE filler while the last normalize chain runs

_PROG_CACHE = {}


def _build_program():
    import concourse.bacc as bacc_mod
    import concourse.mybir as mybir
    import concourse.tile as tile

    f32 = mybir.dt.float32
    f32r = mybir.dt.float32r
    bf16 = mybir.dt.bfloat16
    Exp = mybir.ActivationFunctionType.Exp
    Copy = mybir.ActivationFunctionType.Copy

    nc = bacc_mod.Bacc(
        "TRN2", target_bir_lowering=False, debug=False, num_devices=N_CORES
    )

    xq = nc.dram_tensor("xq", [D_MODEL, S], f32r, kind="ExternalInput").ap()
    xk = nc.dram_tensor("xk", [D_MODEL, S], f32r, kind="ExternalInput").ap()
    xv = nc.dram_tensor("xv", [D_MODEL, S], f32r, kind="ExternalInput").ap()
    wq = nc.dram_tensor("wq", [D_MODEL, 256], f32r, kind="ExternalInput").ap()
    wk = nc.dram_tensor("wk", [D_MODEL, 256], f32r, kind="ExternalInput").ap()
    wv = nc.dram_tensor("wv", [D_MODEL, 256], f32r, kind="ExternalInput").ap()
    wo = nc.dram_tensor("wo", [256, D_MODEL], bf16, kind="ExternalInput").ap()
    maskt = nc.dram_tensor("maskt", [128, 2048], bf16, kind="ExternalInput").ap()
    idbf = nc.dram_tensor("idbf", [128, 132], bf16, kind="ExternalInput").ap()
    y = nc.dram_tensor("y", [S, D_MODEL], f32, kind="ExternalOutput").ap()

    with (
        tile.TileContext(nc) as tc,
        nc.allow_low_precision("bf16/fp32r attention"),
        tc.tile_pool(name="persist", bufs=1) as pp,
        tc.tile_pool(name="xc", bufs=3) as xcp,
        tc.tile_pool(name="et", bufs=6) as etp,
        tc.tile_pool(name="rcsb", bufs=2) as rcp,
        tc.tile_pool(name="bcsb", bufs=2) as bcp,
        tc.tile_pool(name="ysbp", bufs=2) as ysbp,
        tc.tile_pool(name="psS", bufs=2, space="PSUM") as psS,
        tc.tile_pool(name="psO", bufs=2, space="PSUM") as psO,
    ):
        # ---- persistent SBUF tiles ----
        def persist(shape, dtype, name):
            return pp.tile(shape, dtype, name=name, tag=name)

        wq_sb = persist([128, ET * 256], f32r, "wq_sb")
        wk_sb = persist([128, ET * 256], f32r, "wk_sb")
        wv_sb = persist([128, ET * 256], f32r, "wv_sb")
        wo_sb = [persist([128, D_MODEL], bf16, f"wo_sb{p}") for p in range(2)]
        maskt_sb = persist([128, 2048], bf16, "maskt_sb")
        idbf_sb = persist([128, 132], bf16, "idbf_sb")
        # projected Q for the current round, double-buffered by round parity
        qt_blk = [
            [persist([128, 512], bf16, f"qt_blk{par}_{m}") for m in range(2)]
            for par in range(2)
        ]
        kt_sb = [persist([128, S], bf16, f"kt_sb{m}") for m in range(2)]
        v_sb = [persist([128, 260], bf16, f"v_sb{i}") for i in range(KT)]
        outt_sb = [persist([128, S], bf16, f"outt_sb{m}") for m in range(2)]

        # ---- DMA emitters (sync queue; emission order == issue order) ----
        def emit_w_dma(w_dram, w_tile):
            # weight load: [1024, 256] -> [128, 8*256] (e-tile t at cols 256t)
            nc.sync.dma_start(
                out=w_tile[:].rearrange("p (t d) -> p t d", t=ET),
                in_=w_dram.rearrange("(t p) d -> p t d", p=128),
            )

        def emit_x_chunk(x_dram, c, nm):
            # x column chunk: [1024, 512] -> [128, 8*512] (e-tile t at cols 512t)
            t = xcp.tile([128, ET * 512], f32r, name=f"{nm}{c}", tag="xc")
            nc.sync.dma_start(
                out=t[:].rearrange("p (t k) -> p t k", t=ET),
                in_=x_dram[:, c * 512 : (c + 1) * 512].rearrange(
                    "(t p) k -> p t k", p=128
                ),
            )
            return t

        # ---- PE warm-up filler ----
        # The HAM clock gate drops the PE to half clock after any multi-us
        # idle and needs ~3.4us of gapless activity to recover; dummy
        # matmuls (results never read) bridge unavoidable DMA-bound waits.
        def emit_warm(n, name):
            wt = psS.tile([128, 1024], f32, name=name, tag="s")
            for _ in range(n):
                nc.tensor.matmul(
                    wt[:, 0:512], idbf_sb[:, 0:128], maskt_sb[:, 0:512],
                    start=True, stop=True,
                )

        # ---- projection bursts ----
        def emit_proj_qk(c, xt, w_tile, is_q):
            nm = "q" if is_q else "k"
            ps = psS.tile([128, 1024], f32, name=f"psp{nm}_{c}", tag="s")
            for e in range(ET):
                for m in range(2):
                    nc.tensor.matmul(
                        ps[:, m * 512 : (m + 1) * 512],
                        w_tile[:, e * 256 + m * 128 : e * 256 + (m + 1) * 128],
                        xt[:, e * 512 : (e + 1) * 512],
                        start=(e == 0),
                        stop=(e == ET - 1),
                    )
            for m in range(2):
                dst = (
                    qt_blk[c % 2][m][:]
                    if is_q
                    else kt_sb[m][:, c * 512 : (c + 1) * 512]
                )
                nc.vector.tensor_copy(dst, ps[:, m * 512 : (m + 1) * 512])

        def emit_proj_v(c, xt):
            # one PSUM accumulation stream per bank: the matmul start flag
            # clears the whole bank, so two independent 256-wide streams must
            # not share one. Each [128,1024] tile hosts 2 k-tiles at bank
            # starts (cols 0 and 512).
            for half in range(2):
                ps = psS.tile([128, 1024], f32, name=f"pspv_{c}_{half}", tag="s")
                for e in range(ET):
                    for kk in range(2):
                        ktl = 2 * half + kk
                        nc.tensor.matmul(
                            ps[:, kk * 512 : kk * 512 + 256],
                            xt[:, e * 512 + ktl * 128 : e * 512 + (ktl + 1) * 128],
                            wv_sb[:, e * 256 : (e + 1) * 256],
                            start=(e == 0),
                            stop=(e == ET - 1),
                        )
                for kk in range(2):
                    ktl = 2 * half + kk
                    i = 4 * c + ktl
                    nc.vector.tensor_copy(
                        v_sb[i][:].rearrange("p (h c) -> p h c", c=65)[:, :, 0:64],
                        ps[:, kk * 512 : kk * 512 + 256].rearrange(
                            "p (h d) -> p h d", d=64
                        ),
                    )
                    nc.vector.tensor_copy(
                        v_sb[i][:].rearrange("p (h c) -> p h c", c=65)[:, :, 64:65],
                        idbf_sb[:, 128:132].rearrange("p (h c) -> p h c", c=1),
                    )

        # ---- output projection (one 128-row m-tile of y) ----
        def emit_outproj_mtile(m):
            psy = psS.tile([128, 1024], f32, name=f"psy_{m}", tag="s")
            for p in range(2):
                for n in range(2):
                    nc.tensor.matmul(
                        psy[:, n * 512 : (n + 1) * 512],
                        outt_sb[p][:, m * 128 : (m + 1) * 128],
                        wo_sb[p][:, n * 512 : (n + 1) * 512],
                        start=(p == 0),
                        stop=(p == 1),
                    )
            y_sb = ysbp.tile([128, 1024], f32, name=f"y_sb_{m}", tag="ysb")
            # scalar engine stages y out of PSUM: it idles during bursts and
            # this keeps the DVE free for the normalize chain
            nc.scalar.activation(y_sb[:], psy[:], Copy)
            nc.sync.dma_start(out=y[m * 128 : (m + 1) * 128, :], in_=y_sb[:])

        # ---- softmax normalize: no PE involvement ----
        def emit_normalize(pr, jj, ps_out_prev):
            qsj = slice(jj * 512, (jj + 1) * 512)
            ssb = rcp.tile([1, 1024], f32, name=f"ssb_{pr}_{jj}", tag="ssb")
            for hh in range(2):
                nc.vector.tensor_copy(
                    ssb[0:1, 512 * hh : 512 * (hh + 1)],
                    ps_out_prev[64:65, 512 * hh : 512 * (hh + 1)],
                )
            rc = rcp.tile([1, 1024], f32, name=f"rc_{pr}_{jj}", tag="rc")
            nc.vector.reciprocal_approx_fast(out=rc[:], in_=ssb[:])
            bc_sb = bcp.tile([64, 1024], f32, name=f"bc_sb_{pr}_{jj}", tag="bc")
            nc.gpsimd.partition_broadcast(bc_sb[:], rc[0:1, :])
            for hh in range(2):
                nc.vector.tensor_mul(
                    outt_sb[pr][64 * hh : 64 * hh + 64, qsj],
                    ps_out_prev[0:64, 512 * hh : 512 * (hh + 1)],
                    bc_sb[:, 512 * hh : 512 * (hh + 1)],
                )

        # ---- startup: consts + weights + chunk 0, projection 0 ----
        nc.sync.dma_start(out=idbf_sb[:], in_=idbf[:])
        nc.sync.dma_start(out=maskt_sb[:], in_=maskt[:])
        emit_w_dma(wq, wq_sb)
        xq_t = emit_x_chunk(xq, 0, "xq")
        emit_w_dma(wk, wk_sb)
        xk_t = emit_x_chunk(xk, 0, "xk")
        emit_w_dma(wv, wv_sb)
        xv_t = emit_x_chunk(xv, 0, "xv")
        for p in range(2):
            nc.sync.dma_start(out=wo_sb[p][:], in_=wo[p * 128 : (p + 1) * 128, :])

        emit_warm(WARM0, "warm0")
        emit_proj_qk(0, xq_t, wq_sb, True)
        emit_proj_qk(0, xk_t, wk_sb, False)
        emit_proj_v(0, xv_t)
        nxt = (
            emit_x_chunk(xq, 1, "xq"),
            emit_x_chunk(xk, 1, "xk"),
            emit_x_chunk(xv, 1, "xv"),
        )

        # ---- rounds ----
        pending_norm = None  # (pr, j, ps_out) awaiting lazy normalize
        pending_out = []     # m-tiles awaiting output projection

        def emit_attention_block(pr, j):
            nonlocal pending_norm
            n_i = 4 * j + 4
            ps_out = psO.tile([65, 1024], f32, name=f"ps_out_{pr}_{j}", tag="o")
            prev_et = None
            prev_i = -1
            for i in range(n_i):
                diag = i >= 4 * j
                r = i - 4 * j
                pss = psS.tile([128, 1024], f32, name=f"ps_s{pr}_{j}_{i}", tag="s")
                if diag:
                    nw = 128 * (r + 1)
                    for hh in range(2):
                        nc.tensor.matmul(
                            pss[:, 512 * hh : 512 * hh + nw],
                            idbf_sb[:, 0:128],
                            maskt_sb[:, r * 512 : r * 512 + nw],
                            start=True,
                            stop=False,
                        )
                for hh in range(2):
                    hp = slice(64 * hh, 64 * hh + 64)
                    nc.tensor.matmul(
                        pss[:, 512 * hh : 512 * (hh + 1)],
                        kt_sb[pr][hp, i * 128 : (i + 1) * 128],
                        qt_blk[j % 2][pr][hp, :],
                        start=not diag,
                        stop=True,
                    )
                et = etp.tile([128, 1024], bf16, name=f"et{pr}_{j}_{i}", tag="et")
                nc.scalar.activation(et[:], pss[:], Exp, scale=0.125)
                if prev_et is not None:
                    for hh in range(2):
                        nc.tensor.matmul(
                            ps_out[:, 512 * hh : 512 * (hh + 1)],
                            v_sb[prev_i][:, (2 * pr + hh) * 65 : (2 * pr + hh + 1) * 65],
                            prev_et[:, 512 * hh : 512 * (hh + 1)],
                            start=(prev_i == 0),
                            stop=(prev_i == n_i - 1),
                        )
                prev_et, prev_i = et, i
                if i == 1 and pending_norm is not None:
                    pn_pr, pn_j = pending_norm[0], pending_norm[1]
                    emit_normalize(*pending_norm)
                    pending_norm = None
                    if pn_pr == 1:
                        pending_out.extend(range(4 * pn_j, 4 * pn_j + 4))
            for hh in range(2):
                nc.tensor.matmul(
                    ps_out[:, 512 * hh : 512 * (hh + 1)],
                    v_sb[n_i - 1][:, (2 * pr + hh) * 65 : (2 * pr + hh + 1) * 65],
                    prev_et[:, 512 * hh : 512 * (hh + 1)],
                    start=(n_i - 1 == 0),
                    stop=True,
                )
            pending_norm = (pr, j, ps_out)

        for c in range(QT):
            emit_attention_block(0, c)
            # outproj of block c-1 (normalize(pr1,c-1) fired at (pr0,c) i==1)
            while pending_out:
                emit_outproj_mtile(pending_out.pop(0))
            emit_attention_block(1, c)
            if c + 1 < QT:
                emit_proj_qk(c + 1, nxt[0], wq_sb, True)
                emit_proj_qk(c + 1, nxt[1], wk_sb, False)
                emit_proj_v(c + 1, nxt[2])
                if c + 2 < QT:
                    nxt = (
                        emit_x_chunk(xq, c + 2, "xq"),
                        emit_x_chunk(xk, c + 2, "xk"),
                        emit_x_chunk(xv, c + 2, "xv"),
                    )

        # ---- tail: last block's normalize + output projection ----
        emit_warm(WARM_TAIL, "warm_tail")
        emit_normalize(*pending_norm)
        for m in range(4 * (QT - 1), 4 * QT):
            emit_outproj_mtile(m)

    nc.compile()
    return nc


def _get_program():
    if "nc" not in _PROG_CACHE:
        _PROG_CACHE["nc"] = _build_program()
    return _PROG_CACHE["nc"]


def _host_prep(query, key, value, mask, w_q, w_k, w_v, w_o):
    import ml_dtypes

    query = np.asarray(query, dtype=np.float32)
    key = np.asarray(key, dtype=np.float32)
    value = np.asarray(value, dtype=np.float32)
    w_q = np.asarray(w_q, dtype=np.float32)
    w_k = np.asarray(w_k, dtype=np.float32)
    w_v = np.asarray(w_v, dtype=np.float32)
    w_o = np.asarray(w_o, dtype=np.float32)
    m = np.asarray(mask).reshape(S, S).astype(bool)

    # The kernel's block-skip structure assumes the standard causal mask.
    expected = np.triu(np.ones((S, S), dtype=bool), k=1)
    if not np.array_equal(m, expected):
        raise NotImplementedError("kernel specialized for causal (triu, k=1) mask")

    # 4 canonical diagonal-straddle mask tiles: pattern r covers k-tile
    # 4j+r vs q-tile j; masked where (128r + row) > col.
    maskt = np.zeros((128, 2048), dtype=np.float32)
    rows = np.arange(128)[:, None]
    cols = np.arange(512)[None, :]
    for r in range(4):
        maskt[:, r * 512 : (r + 1) * 512] = np.where(
            (128 * r + rows) > cols, np.float32(-1e9), np.float32(0.0)
        )
    maskt = maskt.astype(ml_dtypes.bfloat16)
    idbf = np.zeros((128, 132), dtype=ml_dtypes.bfloat16)
    idbf[:, 0:128] = np.eye(128, dtype=ml_dtypes.bfloat16)
    idbf[:, 128:132] = ml_dtypes.bfloat16(1.0)

    xt = {}
    for b in range(B):
        xt[("q", b)] = np.ascontiguousarray(query[b].T)
        xt[("k", b)] = np.ascontiguousarray(key[b].T)
        xt[("v", b)] = np.ascontiguousarray(value[b].T)

    in_maps = []
    for c in range(N_CORES):
        b = c // 4
        hb = (c % 4) * HPC
        rs = slice(hb * D_K, (hb + HPC) * D_K)
        in_maps.append(
            {
                "xq": xt[("q", b)],
                "xk": xt[("k", b)],
                "xv": xt[("v", b)],
                "wq": np.ascontiguousarray(w_q[rs, :].T),
                "wk": np.ascontiguousarray(w_k[rs, :].T),
                "wv": np.ascontiguousarray(w_v[rs, :].T),
                "wo": np.ascontiguousarray(w_o[:, rs].T).astype(ml_dtypes.bfloat16),
                "maskt": maskt,
                "idbf": idbf,
            }
        )
    return in_maps


def kernel(query, key, value, mask, w_q, w_k, w_v, w_o):
    from concourse.bass_utils import run_bass_kernel_spmd

    in_maps = _host_prep(query, key, value, mask, w_q, w_k, w_v, w_o)
    nc = _get_program()
    res = run_bass_kernel_spmd(nc, in_maps, list(range(N_CORES)))
    out = np.zeros((B, S, D_MODEL), dtype=np.float32)
    for c in range(N_CORES):
        out[c // 4] += res.results[c]["y"]
    return out
